# revision 6
# baseline (speedup 1.0000x reference)
"""LowRankMixtureDensityNetwork loss on 8 Trainium2 NeuronCores.

Data-parallel over the batch (1024 rows/core), MLP weights replicated.
BatchNorm (training mode) statistics are allreduced across cores per layer.
The mixture-density tail uses a bordered 9x9 LDL factorization of
  cap~ = diag(1,..,1,0) + [A|e]^T [A|e]
whose last pivot is the Mahalanobis correction and whose first 8 log-pivots
sum to logdet(cap). Per-core partial loss sums are combined on the host.
"""
import contextlib

import numpy as np

import concourse.bass as bass
import concourse.tile as tile
from concourse import mybir
import bass_rust

F32 = mybir.dt.float32
BF16 = mybir.dt.bfloat16
AF = mybir.ActivationFunctionType
ALU = mybir.AluOpType

# problem constants
DIM, K, RANK = 32, 16, 8
CTX, H, NL, B = 128, 512, 4, 8192
OUT = K + DIM * K + (DIM + DIM * RANK) * K          # 5136
N_CORES = 8
BL = B // N_CORES                                    # 1024 rows per core
NBT = BL // 128                                      # 8 b-tiles per core
NR = RANK + 1                                        # 9 (bordered system)
NRS = NR * (NR + 1) // 2                             # 45 unique (r,s) pairs
LOG2PI = float(np.log(2.0 * np.pi))

# output column regions after host-side permutation of Wout rows:
#   [w(16) | mu(k,d)(512) | diag(k,d)(512) | factor(r,k,d)(4096)]
C_W, C_MU, C_DIAG, C_FAC = 0, K, K + K * DIM, K + 2 * K * DIM

# engine-split knobs
GRAM_GPS_R = (0, 1, 2)     # r-groups whose Gram products+reduces run on GpSimd
CHOL_GPS_MOD = 3           # every CHOL_GPS_MODth chol product op goes to GpSimd


def rs0(r):
    return r * NR - r * (r - 1) // 2


def rs_idx(p, s):
    assert p <= s
    return rs0(p) + (s - p)


# ------------------------------------------------------------- walrus quirks

_ctr = [0]


def _split_multi_waits(nc, max_waits=1):
    """walrus in this container rejects >1 sync wait per instruction; hoist
    excess waits onto same-engine NOPs placed just before the instruction."""
    n_split = 0
    for f in nc.m.functions:
        for bb in f.blocks:
            insts = bb.instructions
            out = []
            changed = False
            for inst in insts:
                si = inst.sync_info
                waits = list(si.on_wait) if si is not None else []
                if len(waits) > max_waits:
                    for w in waits[:-max_waits]:
                        _ctr[0] += 1
                        nop = mybir.InstNoOp(
                            name=f"WSPLIT-{_ctr[0]}",
                            engine=inst.engine,
                            ins=[],
                            outs=[],
                            sync_info=mybir.SyncInfo(on_wait=[w], on_update=[]),
                        )
                        out.append(nop)
                    inst.sync_info = mybir.SyncInfo(
                        on_wait=waits[-max_waits:], on_update=list(si.on_update)
                    )
                    changed = True
                    n_split += 1
                out.append(inst)
            if changed:
                bb.instructions = out
    return n_split


def _patched_drain_and_barrier(self, tick_clock, wait_clock):
    nc = self.nc
    probe = nc.sync.nop()
    wait_clock.add_sem_waits(
        probe.ins, bass_rust.ScopedClock({None: tick_clock.global_clock})
    )
    si = probe.ins.sync_info
    waits = list(si.on_wait) if si is not None else []
    if len(waits) > 1:
        probe.ins.sync_info = mybir.SyncInfo(on_wait=waits[:1], on_update=[])
        for w in waits[1:]:
            extra = nc.sync.nop()
            extra.ins.sync_info = mybir.SyncInfo(on_wait=[w], on_update=[])
    nc.sync.drain()

    nc.all_engine_barrier()
    assert self.sems is not None
    popped = nc._tile_sem_poison_stack.pop()
    assert popped is self._sem_poison
    nc.clear_and_free_semaphores(list(self.sems.allocated().values()))
    nc.all_engine_barrier()


tile.TileContext._drain_and_barrier = _patched_drain_and_barrier


def _bc_mid(ap, n):
    """[P, inner] AP -> [P, n, inner] with a stride-0 middle dim"""
    return bass.AP(tensor=ap.tensor, offset=ap.offset,
                   ap=[ap.ap[0], [0, n], ap.ap[-1]])


def _bc_inner(ap, k):
    """[P, n] AP -> [P, n, k] with a stride-0 inner dim"""
    return bass.AP(tensor=ap.tensor, offset=ap.offset,
                   ap=[ap.ap[0], ap.ap[-1], [0, k]])


# ----------------------------------------------------------------- program


def build_program(split=True):
    nc = bass.Bass("TRN2", num_devices=N_CORES)

    ctxT = nc.dram_tensor("ctxT", [CTX, BL], BF16, kind="ExternalInput")
    data = nc.dram_tensor("data", [NBT, 128, DIM], F32, kind="ExternalInput")
    w0t = nc.dram_tensor("w0t", [CTX, H], BF16, kind="ExternalInput")
    wht = nc.dram_tensor("wht", [NL - 1, H, H], BF16, kind="ExternalInput")
    woutt = nc.dram_tensor("woutt", [H, OUT], BF16, kind="ExternalInput")
    boutr = nc.dram_tensor("boutr", [1, OUT], BF16, kind="ExternalInput")
    # per-feature vectors packed [128, 4hc, 12]:
    #   0:b0 1:g0 2:be0, then per hidden l (0..2): 3+3l:bh, 4+3l:gh, 5+3l:beh
    vecs = nc.dram_tensor("vecs", [128, 4, 12], F32, kind="ExternalInput")
    yout = nc.dram_tensor("yout", [1, 1], F32, kind="ExternalOutput")

    with tile.TileContext(nc) as tc:
        _body(nc, tc, ctxT, data, w0t, wht, woutt, boutr, vecs, yout)
    if split:
        _split_multi_waits(nc)
    return nc


def _body(nc, tc, ctxT, data, w0t, wht, woutt, boutr, vecs, yout):
    ctx = contextlib.ExitStack()
    sb1 = ctx.enter_context(tc.tile_pool(name="persist", bufs=1))
    sbu = ctx.enter_context(tc.tile_pool(name="uacts", bufs=2))
    sbw = ctx.enter_context(tc.tile_pool(name="work", bufs=2))
    sbe = ctx.enter_context(tc.tile_pool(name="elu", bufs=3))
    ps = ctx.enter_context(tc.tile_pool(name="ps", bufs=1, space="PSUM"))
    dram = ctx.enter_context(tc.tile_pool(name="dram", bufs=1, space="DRAM"))

    # ---------------- inputs
    t_ctx = sb1.tile([128, BL], BF16, name="t_ctx")
    nc.sync.dma_start(out=t_ctx[:], in_=ctxT[:])
    t_w0 = sb1.tile([128, H], BF16, name="t_w0")
    nc.sync.dma_start(out=t_w0[:], in_=w0t[:])
    t_wh = sb1.tile([128, NL - 1, 4, H], BF16, name="t_wh")
    nc.sync.dma_start(out=t_wh[:], in_=wht.rearrange("l (c p) m -> p l c m", p=128))
    t_wo = sb1.tile([128, 4, OUT], BF16, name="t_wo")
    nc.sync.dma_start(out=t_wo[:], in_=woutt.rearrange("(c p) m -> p c m", p=128))
    t_bout = sb1.tile([1, OUT], BF16, name="t_bout")
    nc.sync.dma_start(out=t_bout[:], in_=boutr[:])
    t_vec = sb1.tile([128, 4, 12], F32, name="t_vec")
    nc.sync.dma_start(out=t_vec[:], in_=vecs[:])
    t_data = sb1.tile([128, NBT, DIM], F32, name="t_data")
    nc.sync.dma_start(out=t_data[:], in_=data.rearrange("b p d -> p b d"))
    ones1 = sb1.tile([1, 128], BF16, name="ones1")
    nc.vector.memset(ones1[:], 1.0)
    eps_t = sb1.tile([128, 1], F32, name="eps_t")
    nc.vector.memset(eps_t[:], 1e-5)

    # ---------------- collective warmup (absorb first-collective latency)
    cwu_in = dram.tile([128, 1], F32, name="cwu_in")
    cwu_out = dram.tile([128, 1], F32, name="cwu_out")
    t_junk = sb1.tile([128, 1], F32, name="t_junk")
    nc.vector.memset(t_junk[:], 0.0)
    nc.sync.dma_start(out=cwu_in[:], in_=t_junk[:])
    nc.gpsimd.collective_compute(
        "AllReduce", ALU.add, replica_groups=[list(range(N_CORES))],
        ins=[cwu_in[:].opt()], outs=[cwu_out[:].opt()],
    )
    t_junk2 = sb1.tile([128, 1], F32, name="t_junk2")
    nc.gpsimd.dma_start(out=t_junk2[:], in_=cwu_out[:])

    # ---------------- MLP (feature-on-partition)
    u_prev = None
    u3p = None
    wfold = None
    beff = None

    for layer in range(NL):
        u_cur = sbu.tile([128, 4, BL], BF16, name=f"u{layer}", tag="u")
        nkc = 1 if layer == 0 else 4
        for hc in range(4):
            if layer == 0:
                bcol = t_vec[:, hc, 0:1]
            else:
                bcol = beff[:, hc:hc + 1]
            for bcc in range(2):
                bs = bcc * 512
                psum = ps.tile([128, 512], F32, name="zp", tag="z", bufs=3)
                for kc in range(nkc):
                    if layer == 0:
                        lhsT = t_w0[:, hc * 128:(hc + 1) * 128]
                        rhs = t_ctx[:, bs:bs + 512]
                    else:
                        lhsT = wfold[:, kc, hc * 128:(hc + 1) * 128]
                        rhs = u_prev[:, kc, bs:bs + 512]
                    nc.tensor.matmul(psum[:], lhsT=lhsT, rhs=rhs,
                                     start=(kc == 0), stop=(kc == nkc - 1))
                # ELU: u = max(z+b, min(exp(z+b)-1, 0))
                e_t = sbe.tile([128, 512], F32, name="elu_e", tag="elu_e")
                nc.scalar.activation(e_t[:], psum[:], AF.Exp, bias=bcol)
                q2 = sbe.tile([128, 512], BF16, name="elu_q", tag="elu_q")
                nc.vector.tensor_scalar(q2[:], e_t[:], -1.0, 0.0,
                                        op0=ALU.add, op1=ALU.min)
                nc.vector.scalar_tensor_tensor(
                    u_cur[:, hc, bs:bs + 512], psum[:], bcol, q2[:],
                    op0=ALU.add, op1=ALU.max)

        # ---- batch-norm stats (local) -> allreduce -> affine params
        stats = sbw.tile([128, 4, 2, 6], F32, name="bns", tag="bns")
        for hc in range(4):
            for half in range(2):
                nc.vector.bn_stats(
                    out=stats[:, hc, half, :],
                    in_=u_cur[:, hc, half * 512:(half + 1) * 512])
        mv = sbw.tile([128, 4, 2], F32, name="bnmv", tag="bnmv")
        for hc in range(4):
            nc.vector.bn_aggr(out=mv[:, hc, :], in_=stats[:, hc, :, :])
        pack = sbw.tile([128, 8], F32, name="bnp", tag="bnp")
        # s1 = mean * BL
        nc.vector.tensor_scalar_mul(
            pack[:, 0:4], mv[:, :, 0:1].rearrange("p h one -> p (h one)"),
            float(BL))
        # s2 = (var + mean^2) * BL
        msq = sbw.tile([128, 4], F32, name="bmsq", tag="bmsq")
        mm = mv[:, :, 0:1].rearrange("p h one -> p (h one)")
        vv = mv[:, :, 1:2].rearrange("p h one -> p (h one)")
        nc.vector.tensor_tensor(msq[:], mm, mm, op=ALU.mult)
        s2s = sbw.tile([128, 4], F32, name="bs2", tag="bs2")
        nc.vector.tensor_tensor(s2s[:], vv, msq[:], op=ALU.add)
        nc.vector.tensor_scalar_mul(pack[:, 4:8], s2s[:], float(BL))

        ar_in = dram.tile([128, 8], F32, name=f"arin{layer}")
        ar_out = dram.tile([128, 8], F32, name=f"arout{layer}")
        nc.sync.dma_start(out=ar_in[:], in_=pack[:])
        nc.gpsimd.collective_compute(
            "AllReduce", ALU.add, replica_groups=[list(range(N_CORES))],
            ins=[ar_in[:].opt()], outs=[ar_out[:].opt()],
        )
        red = sbw.tile([128, 8], F32, name="bnr", tag="bnr")
        nc.gpsimd.dma_start(out=red[:], in_=ar_out[:])

        iv = 0 if layer == 0 else 3 * (layer - 1) + 3
        g_col = t_vec[:, :, iv + 1]
        be_col = t_vec[:, :, iv + 2]
        m_t = sbw.tile([128, 4], F32, name="bnm", tag="bnm")
        nc.vector.tensor_scalar_mul(m_t[:], red[:, 0:4], 1.0 / B)
        msq2 = sbw.tile([128, 4], F32, name="bnm2", tag="bnm2")
        nc.vector.tensor_tensor(msq2[:], m_t[:], m_t[:], op=ALU.mult)
        var_t = sbw.tile([128, 4], F32, name="bnv", tag="bnv")
        nc.vector.scalar_tensor_tensor(
            var_t[:], red[:, 4:8], 1.0 / B, msq2[:],
            op0=ALU.mult, op1=ALU.subtract)
        # a = g * rsqrt(var+eps) = g * exp(-0.5*ln(var+eps))
        lnv = sbw.tile([128, 4], F32, name="bnl", tag="bnl")
        nc.scalar.activation(lnv[:], var_t[:], AF.Ln, bias=eps_t[:])
        rsq = sbw.tile([128, 4], F32, name="bnq", tag="bnq")
        nc.scalar.activation(rsq[:], lnv[:], AF.Exp, scale=-0.5)
        a_t = sbw.tile([128, 4], F32, name="bna", tag="bna")
        nc.vector.tensor_tensor(a_t[:], g_col, rsq[:], op=ALU.mult)
        ma = sbw.tile([128, 4], F32, name="bnma", tag="bnma")
        nc.vector.tensor_tensor(ma[:], m_t[:], a_t[:], op=ALU.mult)
        c_t = sbw.tile([128, 4], F32, name="bnc", tag="bnc")
        nc.vector.tensor_tensor(c_t[:], be_col, ma[:], op=ALU.subtract)

        if layer < NL - 1:
            # fold affine into next layer: W' = WhT * a (per contraction row)
            wfold = sbw.tile([128, 4, H], BF16, name="wf", tag="wf")
            for kc in range(4):
                nc.vector.tensor_scalar_mul(
                    wfold[:, kc, :], t_wh[:, layer, kc, :], a_t[:, kc:kc + 1])
            # beff = Wh^T... bias: z_{l+1} = W'u + (Wh[layer] @ c + b_{l+1})
            c_bf = sbw.tile([128, 4], BF16, name="cbf", tag="cbf")
            nc.vector.tensor_copy(c_bf[:], c_t[:])
            beff = sbw.tile([128, 4], F32, name="beff", tag="beff")
            b_next = t_vec[:, :, 3 * layer + 3]  # bh[layer] columns [128,4]
            for mc in range(4):
                pb = ps.tile([128, 1], F32, name="pbias", tag="pbias", bufs=1)
                for kc in range(4):
                    nc.tensor.matmul(
                        pb[:],
                        lhsT=t_wh[:, layer, kc, mc * 128:(mc + 1) * 128],
                        rhs=c_bf[:, kc:kc + 1],
                        start=(kc == 0), stop=(kc == 3))
                nc.scalar.activation(
                    beff[:, mc:mc + 1], pb[:], AF.Identity,
                    bias=b_next[:, mc:mc + 1])
            u_prev = u_cur
        else:
            # BN3 applied directly on u (Wout stays raw)
            u3p = sb1.tile([128, 4, BL], BF16, name="u3p")
            for hc in range(4):
                nc.scalar.activation(
                    u3p[:, hc, :], u_cur[:, hc, :], AF.Identity,
                    bias=c_t[:, hc:hc + 1], scale=a_t[:, hc:hc + 1])

    # ---------------- output layer + mixture tail (batch-on-partition)
    capG = sb1.tile([128, NRS, NBT, K], BF16, name="capG")
    ldall = sb1.tile([128, NBT, K], F32, name="ldall")
    wall = sb1.tile([128, NBT, K], F32, name="wall")

    chunks = [(C_W, K, "w"), (C_MU, K * DIM, "mu"), (C_DIAG, K * DIM, "diag")]
    chunks += [(C_FAC + r * 512, 512, f"fac{r}") for r in range(RANK)]

    for bt in range(NBT):
        bts = bt * 128
        s_f = sbw.tile([128, K * DIM], F32, name="s_f", tag="s_f")
        s_bf = sbw.tile([128, K * DIM], BF16, name="s_bf", tag="s_bf")
        diff = sbw.tile([128, K * DIM], F32, name="diff", tag="diff")
        At = sbw.tile([128, NR, 512], BF16, name="At", tag="At")

        for c0, w, kind in chunks:
            psum = ps.tile([128, w], F32, name=f"po_{kind}", tag="po", bufs=3)
            nc.tensor.matmul(psum[:], lhsT=ones1[:], rhs=t_bout[:, c0:c0 + w],
                             start=True, stop=False)
            for kc in range(4):
                nc.tensor.matmul(
                    psum[:], lhsT=u3p[:, kc, bts:bts + 128],
                    rhs=t_wo[:, kc, c0:c0 + w],
                    start=False, stop=(kc == 3))
            if kind == "w":
                nc.scalar.copy(wall[:, bt, :], psum[:])
            elif kind == "mu":
                nc.vector.tensor_tensor(
                    diff[:], _bc_mid(t_data[:, bt, :], K), psum[:],
                    op=ALU.subtract)
            elif kind == "diag":
                nc.scalar.activation(s_f[:], psum[:], AF.Exp, scale=-0.5)
                nc.vector.tensor_reduce(
                    out=ldall[:, bt, :],
                    in_=psum[:].rearrange("p (k d) -> p k d", d=DIM),
                    axis=mybir.AxisListType.X, op=ALU.add)
                nc.vector.tensor_copy(s_bf[:], s_f[:])
            else:
                r = int(kind[3:])
                nc.vector.tensor_tensor(
                    At[:, r, :], psum[:], s_bf[:], op=ALU.mult)
        nc.vector.tensor_tensor(At[:, RANK, :], diff[:], s_f[:], op=ALU.mult)

        # Gram products + segmented reduces, grouped by leading index r
        for r in range(NR):
            nsr = NR - r
            eng = nc.gpsimd if r in GRAM_GPS_R else nc.vector
            pscr = sbw.tile([128, nsr, 512], BF16, name="pscr", tag="pscr")
            eng.tensor_tensor(
                pscr[:], _bc_mid(At[:, r, :], nsr), At[:, r:NR, :],
                op=ALU.mult)
            redscr = sbw.tile([128, nsr * K], F32, name="redscr", tag="redscr")
            nc.vector.tensor_reduce(
                out=redscr[:],
                in_=pscr[:].rearrange("p s (k d) -> p (s k) d", d=DIM),
                axis=mybir.AxisListType.X, op=ALU.add)
            nc.vector.tensor_copy(
                capG[:, rs0(r):rs0(r) + nsr, bt, :],
                redscr[:].rearrange("p (s k) -> p s k", k=K))

    for j in range(RANK):
        nc.vector.tensor_scalar_add(
            capG[:, rs_idx(j, j), :, :], capG[:, rs_idx(j, j), :, :], 1.0)

    # ---------------- bordered LDL over [128, (bt,k)] planes
    Lbf = sb1.tile([128, NRS, NBT * K], BF16, name="Lbf")
    pivd = sb1.tile([128, NR, NBT * K], F32, name="pivd")

    def g_plane(i, j):
        return capG[:, rs_idx(j, i), :, :].rearrange("p b k -> p (b k)")

    cnt = [0]

    def chol_eng():
        cnt[0] += 1
        if CHOL_GPS_MOD and cnt[0] % CHOL_GPS_MOD == 0:
            return nc.gpsimd
        return nc.vector

    inv_cur = None
    for j in range(NR):
        for i in range(j, NR):
            prodscr = sbw.tile([128, max(j, 1), NBT * K], BF16,
                               name="prodscr", tag="prodscr")
            terms = []
            for p in range(j):
                chol_eng().tensor_tensor(
                    prodscr[:, p, :], Lbf[:, rs_idx(p, i), :],
                    g_plane(j, p), op=ALU.mult)
                terms.append(p)
            # pairwise tree-sum into terms[0]
            while len(terms) > 1:
                nxt = []
                for q in range(0, len(terms) - 1, 2):
                    a0, a1 = terms[q], terms[q + 1]
                    chol_eng().tensor_tensor(
                        prodscr[:, a0, :], prodscr[:, a0, :],
                        prodscr[:, a1, :], op=ALU.add)
                    nxt.append(a0)
                if len(terms) % 2 == 1:
                    nxt.append(terms[-1])
                terms = nxt
            tgt = pivd[:, j, :] if i == j else g_plane(i, j)
            if j == 0:
                if i == j:
                    nc.vector.tensor_copy(tgt, g_plane(i, j))
                # off-diag V_i0 == G_i0 already in place
            else:
                nc.vector.tensor_tensor(
                    tgt, g_plane(i, j), prodscr[:, terms[0], :],
                    op=ALU.subtract)
            if i == j:
                if j < NR - 1:
                    inv_cur = sbw.tile([128, NBT * K], F32,
                                       name="invj", tag="invj")
                    nc.vector.reciprocal(inv_cur[:], pivd[:, j, :])
            else:
                nc.vector.tensor_tensor(
                    Lbf[:, rs_idx(j, i), :], g_plane(i, j),
                    inv_cur[:], op=ALU.mult)

    # ---------------- comp_logp, double logsumexp, local sum
    ldt = sbw.tile([128, NBT * K], F32, name="ldt", tag="ldt")
    nc.vector.tensor_copy(ldt[:], ldall[:].rearrange("p b k -> p (b k)"))
    lnp = sbw.tile([128, NBT * K], F32, name="lnp", tag="lnp")
    for j in range(RANK):
        nc.scalar.activation(lnp[:], pivd[:, j, :], AF.Ln)
        nc.vector.tensor_tensor(ldt[:], ldt[:], lnp[:], op=ALU.add)
    comp = sbw.tile([128, NBT * K], F32, name="comp", tag="comp")
    nc.vector.tensor_tensor(comp[:], ldt[:], pivd[:, NR - 1, :], op=ALU.add)
    nc.vector.tensor_scalar(comp[:], comp[:], float(DIM * LOG2PI), -0.5,
                            op0=ALU.add, op1=ALU.mult)

    # log_prob = logsumexp_k(w + comp) - logsumexp_k(w)
    t_t = sbw.tile([128, NBT, K], F32, name="t_t", tag="t_t")
    nc.vector.tensor_tensor(
        t_t[:], wall[:], comp[:].rearrange("p (b k) -> p b k", k=K),
        op=ALU.add)

    def lse_k(src3d, nm):
        mx = sbw.tile([128, NBT], F32, name=f"mx{nm}", tag=f"mx{nm}")
        nc.vector.tensor_reduce(out=mx[:], in_=src3d,
                                axis=mybir.AxisListType.X, op=ALU.max)
        zs = sbw.tile([128, NBT, K], F32, name=f"zs{nm}", tag=f"zs{nm}")
        nc.vector.tensor_tensor(zs[:], src3d, _bc_inner(mx[:], K),
                                op=ALU.subtract)
        ez = sbw.tile([128, NBT, K], F32, name=f"ez{nm}", tag=f"ez{nm}")
        nc.scalar.activation(ez[:], zs[:], AF.Exp)
        sez = sbw.tile([128, NBT], F32, name=f"se{nm}", tag=f"se{nm}")
        nc.vector.tensor_reduce(out=sez[:], in_=ez[:],
                                axis=mybir.AxisListType.X, op=ALU.add)
        ls = sbw.tile([128, NBT], F32, name=f"ls{nm}", tag=f"ls{nm}")
        nc.scalar.activation(ls[:], sez[:], AF.Ln)
        out = sbw.tile([128, NBT], F32, name=f"lo{nm}", tag=f"lo{nm}")
        nc.vector.tensor_tensor(out[:], mx[:], ls[:], op=ALU.add)
        return out

    lp1 = lse_k(t_t[:], "t")
    lpw = lse_k(wall[:], "w")
    lp = sbw.tile([128, NBT], F32, name="lp", tag="lp")
    nc.vector.tensor_tensor(lp[:], lp1[:], lpw[:], op=ALU.subtract)

    lps = sbw.tile([128, 1], F32, name="lps", tag="lps")
    nc.vector.tensor_reduce(out=lps[:], in_=lp[:],
                            axis=mybir.AxisListType.X, op=ALU.add)
    ones_f = sb1.tile([128, 1], F32, name="ones_f")
    nc.vector.memset(ones_f[:], 1.0)
    pfin = ps.tile([1, 1], F32, name="pfin", tag="pfin", bufs=1)
    nc.tensor.matmul(pfin[:], lhsT=lps[:], rhs=ones_f[:], start=True, stop=True)
    yt = sbw.tile([1, 1], F32, name="yt", tag="yt")
    nc.scalar.copy(yt[:], pfin[:])
    nc.sync.dma_start(out=yout[:], in_=yt[:])

    ctx.close()


# --------------------------------------------------------------- host side

_CACHE = {}


def _perm():
    idx_w = np.arange(K)
    idx_mu = K + np.arange(K * DIM)
    base = K + K * DIM
    idx_diag = np.empty((K, DIM), np.int64)
    idx_fac = np.empty((RANK, K, DIM), np.int64)
    for k in range(K):
        blk = base + k * (DIM + DIM * RANK)
        idx_diag[k] = blk + np.arange(DIM)
        for d in range(DIM):
            for r in range(RANK):
                idx_fac[r, k, d] = blk + DIM + d * RANK + r
    return np.concatenate([idx_w, idx_mu, idx_diag.ravel(), idx_fac.ravel()])


def _prep(inputs):
    import ml_dtypes
    bf = ml_dtypes.bfloat16
    perm = _perm()
    Wp = np.asarray(inputs["Wout"], np.float32)[perm]
    bp = np.asarray(inputs["bout"], np.float32)[perm][None, :].astype(bf)
    w0t = np.ascontiguousarray(np.asarray(inputs["W0"], np.float32).T).astype(bf)
    wht = np.ascontiguousarray(
        np.transpose(np.asarray(inputs["Wh"], np.float32), (0, 2, 1))).astype(bf)
    woutt = np.ascontiguousarray(Wp.T).astype(bf)

    def v128(v):
        return np.ascontiguousarray(np.asarray(v, np.float32).reshape(4, 128).T)

    vec_list = [inputs["b0"], inputs["g0"], inputs["be0"]]
    for li in range(NL - 1):
        vec_list += [inputs["bh"][li], inputs["gh"][li], inputs["beh"][li]]
    vecs = np.stack([v128(v) for v in vec_list], axis=-1).astype(np.float32)

    data = np.asarray(inputs["data"], np.float32)
    context = np.asarray(inputs["context"], np.float32)
    in_maps = []
    for c in range(N_CORES):
        sl = slice(c * BL, (c + 1) * BL)
        in_maps.append({
            "ctxT": np.ascontiguousarray(context[sl].T).astype(bf),
            "data": np.ascontiguousarray(data[sl].reshape(NBT, 128, DIM)),
            "w0t": w0t, "wht": wht, "woutt": woutt, "boutr": bp, "vecs": vecs,
        })
    return in_maps


def kernel(**inputs):
    from concourse.bass_utils import run_bass_kernel_spmd

    if "nc" not in _CACHE:
        _CACHE["nc"] = build_program()
    nc = _CACHE["nc"]
    in_maps = _prep(inputs)
    res = run_bass_kernel_spmd(nc, in_maps, core_ids=list(range(N_CORES)))
    total = sum(float(res.results[c]["yout"][0, 0]) for c in range(N_CORES))
    return np.float32(-total / B)


# revision 13
# speedup vs baseline: 1.0481x; 1.0481x over previous
"""LowRankMixtureDensityNetwork loss on 8 Trainium2 NeuronCores.

Data-parallel over the batch (1024 rows/core), MLP weights replicated.
BatchNorm (training mode) statistics are allreduced across cores per layer.
The mixture-density tail uses a bordered 9x9 LDL factorization of
  cap~ = diag(1,..,1,0) + [A|e]^T [A|e]
whose last pivot is the Mahalanobis correction and whose first 8 log-pivots
sum to logdet(cap). Per-core partial loss sums are combined on the host.

Layout notes:
- MLP runs feature-on-partition; the output layer flips to batch-on-partition
  by using the activations as the matmul's stationary operand.
- Gram products are batched by diagonal offset o (pairs (r, r+o)) so both
  operands are dense slices of At; the d-reduction is a bf16 fold tree
  (tensor_tensor runs 2x on bf16, tensor_reduce is capped at 1x).
- cap~ is stored on a 9x9=81-slot grid: diagonal writes stride 10, column
  slabs stride 1 - all constant-stride APs.
"""
import contextlib

import numpy as np

import concourse.bass as bass
import concourse.tile as tile
from concourse import mybir
import bass_rust

F32 = mybir.dt.float32
BF16 = mybir.dt.bfloat16
AF = mybir.ActivationFunctionType
ALU = mybir.AluOpType

# problem constants
DIM, K, RANK = 32, 16, 8
CTX, H, NL, B = 128, 512, 4, 8192
OUT = K + DIM * K + (DIM + DIM * RANK) * K          # 5136
N_CORES = 8
BL = B // N_CORES                                    # 1024 rows per core
NBT = BL // 128                                      # 8 b-tiles per core
BK = NBT * K                                         # 128 (bt,k) plane width
NR = RANK + 1                                        # 9 (bordered system)
LOG2PI = float(np.log(2.0 * np.pi))

# output column regions after host-side permutation of Wout rows:
#   [w(16) | mu(k,d)(512) | diag(k,d)(512) | factor(r,k,d)(4096)]
C_W, C_MU, C_DIAG, C_FAC = 0, K, K + K * DIM, K + 2 * K * DIM

# engine-split knobs
GRAM_GPS_O = (4, 5, 6, 7, 8)   # Gram diagonals handled by GpSimd
ABUILD_GPS_R = (4, 5, 6, 7)    # A-build rows multiplied on GpSimd

# ------------------------------------------------------------- walrus quirks

_ctr = [0]


def _split_multi_waits(nc, max_waits=1):
    """walrus in this container rejects >1 sync wait per instruction; hoist
    excess waits onto same-engine NOPs placed just before the instruction."""
    n_split = 0
    for f in nc.m.functions:
        for bb in f.blocks:
            insts = bb.instructions
            out = []
            changed = False
            for inst in insts:
                si = inst.sync_info
                waits = list(si.on_wait) if si is not None else []
                if len(waits) > max_waits:
                    for w in waits[:-max_waits]:
                        _ctr[0] += 1
                        nop = mybir.InstNoOp(
                            name=f"WSPLIT-{_ctr[0]}",
                            engine=inst.engine,
                            ins=[],
                            outs=[],
                            sync_info=mybir.SyncInfo(on_wait=[w], on_update=[]),
                        )
                        out.append(nop)
                    inst.sync_info = mybir.SyncInfo(
                        on_wait=waits[-max_waits:], on_update=list(si.on_update)
                    )
                    changed = True
                    n_split += 1
                out.append(inst)
            if changed:
                bb.instructions = out
    return n_split


def _patched_drain_and_barrier(self, tick_clock, wait_clock):
    nc = self.nc
    probe = nc.sync.nop()
    wait_clock.add_sem_waits(
        probe.ins, bass_rust.ScopedClock({None: tick_clock.global_clock})
    )
    si = probe.ins.sync_info
    waits = list(si.on_wait) if si is not None else []
    if len(waits) > 1:
        probe.ins.sync_info = mybir.SyncInfo(on_wait=waits[:1], on_update=[])
        for w in waits[1:]:
            extra = nc.sync.nop()
            extra.ins.sync_info = mybir.SyncInfo(on_wait=[w], on_update=[])
    nc.sync.drain()

    nc.all_engine_barrier()
    assert self.sems is not None
    popped = nc._tile_sem_poison_stack.pop()
    assert popped is self._sem_poison
    nc.clear_and_free_semaphores(list(self.sems.allocated().values()))
    nc.all_engine_barrier()


tile.TileContext._drain_and_barrier = _patched_drain_and_barrier


def _bc_mid(ap, n):
    """[P, inner] AP -> [P, n, inner] with a stride-0 middle dim"""
    return bass.AP(tensor=ap.tensor, offset=ap.offset,
                   ap=[ap.ap[0], [0, n], ap.ap[-1]])


def _bc_inner(ap, k):
    """[P, n] AP -> [P, n, k] with a stride-0 inner dim"""
    return bass.AP(tensor=ap.tensor, offset=ap.offset,
                   ap=[ap.ap[0], ap.ap[-1], [0, k]])


# ----------------------------------------------------------------- program


def build_program(split=True):
    nc = bass.Bass("TRN2", num_devices=N_CORES)

    ctxT = nc.dram_tensor("ctxT", [CTX, BL], BF16, kind="ExternalInput")
    data = nc.dram_tensor("data", [NBT, 128, DIM], F32, kind="ExternalInput")
    w0t = nc.dram_tensor("w0t", [CTX, H], BF16, kind="ExternalInput")
    wht = nc.dram_tensor("wht", [NL - 1, H, H], BF16, kind="ExternalInput")
    woutt = nc.dram_tensor("woutt", [H, OUT], BF16, kind="ExternalInput")
    boutr = nc.dram_tensor("boutr", [1, OUT], BF16, kind="ExternalInput")
    # per-feature vectors packed [128, 4hc, 12]:
    #   0:b0 1:g0 2:be0, then per hidden l (0..2): 3+3l:bh, 4+3l:gh, 5+3l:beh
    vecs = nc.dram_tensor("vecs", [128, 4, 12], F32, kind="ExternalInput")
    yout = nc.dram_tensor("yout", [1, 1], F32, kind="ExternalOutput")

    with tile.TileContext(nc) as tc:
        _body(nc, tc, ctxT, data, w0t, wht, woutt, boutr, vecs, yout)
    if split:
        _split_multi_waits(nc)
    return nc


def _mlp(nc, tc, sb1, ps, dram, ctxT, w0t, wht, vecs):
    """feature-on-partition MLP with cross-core BN; returns u3p (bf16)."""
    ctx = contextlib.ExitStack()
    sbm = ctx.enter_context(tc.tile_pool(name="mlpwork", bufs=2))
    sbu = ctx.enter_context(tc.tile_pool(name="uacts", bufs=2))
    sbe = ctx.enter_context(tc.tile_pool(name="elu", bufs=3))

    t_ctx = sbm.tile([128, BL], BF16, name="t_ctx", tag="t_ctx", bufs=1)
    nc.sync.dma_start(out=t_ctx[:], in_=ctxT[:])
    t_w0 = sbm.tile([128, H], BF16, name="t_w0", tag="t_w0", bufs=1)
    nc.sync.dma_start(out=t_w0[:], in_=w0t[:])
    t_wh = sbm.tile([128, NL - 1, 4, H], BF16, name="t_wh", tag="t_wh", bufs=1)
    nc.sync.dma_start(out=t_wh[:], in_=wht.rearrange("l (c p) m -> p l c m", p=128))
    t_vec = sbm.tile([128, 4, 12], F32, name="t_vec", tag="t_vec", bufs=1)
    nc.sync.dma_start(out=t_vec[:], in_=vecs[:])
    eps_t = sbm.tile([128, 1], F32, name="eps_t", tag="eps_t", bufs=1)
    nc.vector.memset(eps_t[:], 1e-5)

    # collective warmup (absorb first-collective latency)
    cwu_in = dram.tile([128, 1], F32, name="cwu_in")
    cwu_out = dram.tile([128, 1], F32, name="cwu_out")
    t_junk = sbm.tile([128, 1], F32, name="t_junk", tag="t_junk", bufs=1)
    nc.vector.memset(t_junk[:], 0.0)
    nc.sync.dma_start(out=cwu_in[:], in_=t_junk[:])
    nc.gpsimd.collective_compute(
        "AllReduce", ALU.add, replica_groups=[list(range(N_CORES))],
        ins=[cwu_in[:].opt()], outs=[cwu_out[:].opt()],
    )
    t_junk2 = sbm.tile([128, 1], F32, name="t_junk2", tag="t_junk2", bufs=1)
    nc.gpsimd.dma_start(out=t_junk2[:], in_=cwu_out[:])

    u_prev = None
    u3p = None
    wfold = None
    beff = None

    for layer in range(NL):
        u_cur = sbu.tile([128, 4, BL], BF16, name=f"u{layer}", tag="u")
        nkc = 1 if layer == 0 else 4
        for hc in range(4):
            if layer == 0:
                bcol = t_vec[:, hc, 0:1]
            else:
                bcol = beff[:, hc:hc + 1]
            for bcc in range(2):
                bs = bcc * 512
                psum = ps.tile([128, 512], F32, name="zp", tag="z", bufs=3)
                for kc in range(nkc):
                    if layer == 0:
                        lhsT = t_w0[:, hc * 128:(hc + 1) * 128]
                        rhs = t_ctx[:, bs:bs + 512]
                    else:
                        lhsT = wfold[:, kc, hc * 128:(hc + 1) * 128]
                        rhs = u_prev[:, kc, bs:bs + 512]
                    nc.tensor.matmul(psum[:], lhsT=lhsT, rhs=rhs,
                                     start=(kc == 0), stop=(kc == nkc - 1))
                # ELU: u = max(z+b, min(exp(z+b)-1, 0))
                e_t = sbe.tile([128, 512], F32, name="elu_e", tag="elu_e")
                nc.scalar.activation(e_t[:], psum[:], AF.Exp, bias=bcol)
                q2 = sbe.tile([128, 512], BF16, name="elu_q", tag="elu_q")
                nc.vector.tensor_scalar(q2[:], e_t[:], -1.0, 0.0,
                                        op0=ALU.add, op1=ALU.min)
                nc.vector.scalar_tensor_tensor(
                    u_cur[:, hc, bs:bs + 512], psum[:], bcol, q2[:],
                    op0=ALU.add, op1=ALU.max)

        # ---- batch-norm stats (local) -> allreduce -> affine params
        stats = sbm.tile([128, 4, 2, 6], F32, name="bns", tag="bns")
        for hc in range(4):
            for half in range(2):
                nc.vector.bn_stats(
                    out=stats[:, hc, half, :],
                    in_=u_cur[:, hc, half * 512:(half + 1) * 512])
        mv = sbm.tile([128, 4, 2], F32, name="bnmv", tag="bnmv")
        for hc in range(4):
            nc.vector.bn_aggr(out=mv[:, hc, :], in_=stats[:, hc, :, :])
        pack = sbm.tile([128, 8], F32, name="bnp", tag="bnp")
        mm = mv[:, :, 0:1].rearrange("p h one -> p (h one)")
        vv = mv[:, :, 1:2].rearrange("p h one -> p (h one)")
        nc.vector.tensor_scalar_mul(pack[:, 0:4], mm, float(BL))
        msq = sbm.tile([128, 4], F32, name="bmsq", tag="bmsq")
        nc.vector.tensor_tensor(msq[:], mm, mm, op=ALU.mult)
        s2s = sbm.tile([128, 4], F32, name="bs2", tag="bs2")
        nc.vector.tensor_tensor(s2s[:], vv, msq[:], op=ALU.add)
        nc.vector.tensor_scalar_mul(pack[:, 4:8], s2s[:], float(BL))

        ar_in = dram.tile([128, 8], F32, name=f"arin{layer}")
        ar_out = dram.tile([128, 8], F32, name=f"arout{layer}")
        nc.sync.dma_start(out=ar_in[:], in_=pack[:])
        nc.gpsimd.collective_compute(
            "AllReduce", ALU.add, replica_groups=[list(range(N_CORES))],
            ins=[ar_in[:].opt()], outs=[ar_out[:].opt()],
        )
        red = sbm.tile([128, 8], F32, name="bnr", tag="bnr")
        nc.gpsimd.dma_start(out=red[:], in_=ar_out[:])

        iv = 0 if layer == 0 else 3 * (layer - 1) + 3
        g_col = t_vec[:, :, iv + 1]
        be_col = t_vec[:, :, iv + 2]
        m_t = sbm.tile([128, 4], F32, name="bnm", tag="bnm")
        nc.vector.tensor_scalar_mul(m_t[:], red[:, 0:4], 1.0 / B)
        msq2 = sbm.tile([128, 4], F32, name="bnm2", tag="bnm2")
        nc.vector.tensor_tensor(msq2[:], m_t[:], m_t[:], op=ALU.mult)
        var_t = sbm.tile([128, 4], F32, name="bnv", tag="bnv")
        nc.vector.scalar_tensor_tensor(
            var_t[:], red[:, 4:8], 1.0 / B, msq2[:],
            op0=ALU.mult, op1=ALU.subtract)
        # a = g * rsqrt(var+eps) = g * exp(-0.5*ln(var+eps))
        lnv = sbm.tile([128, 4], F32, name="bnl", tag="bnl")
        nc.scalar.activation(lnv[:], var_t[:], AF.Ln, bias=eps_t[:])
        rsq = sbm.tile([128, 4], F32, name="bnq", tag="bnq")
        nc.scalar.activation(rsq[:], lnv[:], AF.Exp, scale=-0.5)
        a_t = sbm.tile([128, 4], F32, name="bna", tag="bna")
        nc.vector.tensor_tensor(a_t[:], g_col, rsq[:], op=ALU.mult)
        ma = sbm.tile([128, 4], F32, name="bnma", tag="bnma")
        nc.vector.tensor_tensor(ma[:], m_t[:], a_t[:], op=ALU.mult)
        c_t = sbm.tile([128, 4], F32, name="bnc", tag="bnc")
        nc.vector.tensor_tensor(c_t[:], be_col, ma[:], op=ALU.subtract)

        if layer < NL - 1:
            # fold affine into next layer: W' = WhT * a (per contraction row)
            wfold = sbm.tile([128, 4, H], BF16, name="wf", tag="wf")
            for kc in range(4):
                nc.vector.tensor_scalar_mul(
                    wfold[:, kc, :], t_wh[:, layer, kc, :], a_t[:, kc:kc + 1])
            # bias: z_{l+1} = W'u + (Wh[layer] @ c + b_{l+1})
            c_bf = sbm.tile([128, 4], BF16, name="cbf", tag="cbf")
            nc.vector.tensor_copy(c_bf[:], c_t[:])
            beff = sbm.tile([128, 4], F32, name="beff", tag="beff")
            b_next = t_vec[:, :, 3 * layer + 3]
            for mc in range(4):
                pb = ps.tile([128, 1], F32, name="pbias", tag="pbias", bufs=1)
                for kc in range(4):
                    nc.tensor.matmul(
                        pb[:],
                        lhsT=t_wh[:, layer, kc, mc * 128:(mc + 1) * 128],
                        rhs=c_bf[:, kc:kc + 1],
                        start=(kc == 0), stop=(kc == 3))
                nc.scalar.activation(
                    beff[:, mc:mc + 1], pb[:], AF.Identity,
                    bias=b_next[:, mc:mc + 1])
            u_prev = u_cur
        else:
            # BN3 applied directly on u (Wout stays raw)
            u3p = sb1.tile([128, 4, BL], BF16, name="u3p")
            for hc in range(4):
                nc.scalar.activation(
                    u3p[:, hc, :], u_cur[:, hc, :], AF.Identity,
                    bias=c_t[:, hc:hc + 1], scale=a_t[:, hc:hc + 1])

    ctx.close()
    return u3p


def _body(nc, tc, ctxT, data, w0t, wht, woutt, boutr, vecs, yout):
    ctx = contextlib.ExitStack()
    sb1 = ctx.enter_context(tc.tile_pool(name="persist", bufs=1))
    ps = ctx.enter_context(tc.tile_pool(name="ps", bufs=1, space="PSUM"))
    dram = ctx.enter_context(tc.tile_pool(name="dram", bufs=1, space="DRAM"))

    t_wo = sb1.tile([128, 4, OUT], BF16, name="t_wo")
    nc.sync.dma_start(out=t_wo[:], in_=woutt.rearrange("(c p) m -> p c m", p=128))
    t_bout = sb1.tile([1, OUT], BF16, name="t_bout")
    nc.sync.dma_start(out=t_bout[:], in_=boutr[:])
    t_data = sb1.tile([128, NBT, DIM], F32, name="t_data")
    nc.sync.dma_start(out=t_data[:], in_=data.rearrange("b p d -> p b d"))
    ones1 = sb1.tile([1, 128], BF16, name="ones1")
    nc.vector.memset(ones1[:], 1.0)

    u3p = _mlp(nc, tc, sb1, ps, dram, ctxT, w0t, wht, vecs)

    # ---------------- output layer + mixture tail (batch-on-partition)
    sbt = ctx.enter_context(tc.tile_pool(name="tail", bufs=2))
    # cap~ on the 81-slot grid: entry (a,b), a<=b, at slot a*9+b
    capG = sb1.tile([128, 81, NBT, K], BF16, name="capG")
    ldall = sb1.tile([128, NBT, K], F32, name="ldall")
    wall = sb1.tile([128, NBT, K], F32, name="wall")

    chunks = [(C_W, K, "w"), (C_MU, K * DIM, "mu"), (C_DIAG, K * DIM, "diag")]
    chunks += [(C_FAC + r * 512, 512, f"fac{r}") for r in range(RANK)]

    FW = 16 + 8 + 4 + 2   # fold scratch cols per (pair,k): L1..L4 outputs

    for bt in range(NBT):
        bts = bt * 128
        s_f = sbt.tile([128, K * DIM], F32, name="s_f", tag="s_f")
        s_bf = sbt.tile([128, K * DIM], BF16, name="s_bf", tag="s_bf")
        diff = sbt.tile([128, K * DIM], F32, name="diff", tag="diff")
        At = sbt.tile([128, NR, 512], BF16, name="At", tag="At")

        for c0, w, kind in chunks:
            psum = ps.tile([128, w], F32, name=f"po_{kind}", tag="po", bufs=4)
            nc.tensor.matmul(psum[:], lhsT=ones1[:], rhs=t_bout[:, c0:c0 + w],
                             start=True, stop=False)
            for kc in range(4):
                nc.tensor.matmul(
                    psum[:], lhsT=u3p[:, kc, bts:bts + 128],
                    rhs=t_wo[:, kc, c0:c0 + w],
                    start=False, stop=(kc == 3))
            if kind == "w":
                nc.scalar.copy(wall[:, bt, :], psum[:])
            elif kind == "mu":
                nc.vector.tensor_tensor(
                    diff[:], _bc_mid(t_data[:, bt, :], K), psum[:],
                    op=ALU.subtract)
            elif kind == "diag":
                nc.scalar.activation(s_f[:], psum[:], AF.Exp, scale=-0.5)
                nc.vector.tensor_reduce(
                    out=ldall[:, bt, :],
                    in_=psum[:].rearrange("p (k d) -> p k d", d=DIM),
                    axis=mybir.AxisListType.X, op=ALU.add)
                nc.vector.tensor_copy(s_bf[:], s_f[:])
            else:
                r = int(kind[3:])
                # F evac on ACT (bf16), multiply by s on DVE/GPS
                fbf = sbt.tile([128, 512], BF16, name="fbf", tag="fbf", bufs=3)
                nc.scalar.copy(fbf[:], psum[:])
                eng = nc.gpsimd if r in ABUILD_GPS_R else nc.vector
                eng.tensor_tensor(At[:, r, :], fbf[:], s_bf[:], op=ALU.mult)
        nc.vector.tensor_tensor(At[:, RANK, :], diff[:], s_f[:], op=ALU.mult)

        # Gram by diagonal offset o: pairs (r, r+o), both operands dense
        for o in range(NR):
            n = NR - o
            gps = o in GRAM_GPS_O
            eng = nc.gpsimd if gps else nc.vector
            sfx = "g" if gps else "d"
            pscr = sbt.tile([128, n, 512], BF16, name=f"pscr{sfx}",
                            tag=f"pscr{sfx}")
            eng.tensor_tensor(pscr[:], At[:, 0:n, :], At[:, o:NR, :],
                              op=ALU.mult)
            pv = pscr[:].rearrange("p n (k d) -> p (n k) d", d=DIM)
            fs = sbt.tile([128, n * K, FW], BF16, name=f"fs{sfx}",
                          tag=f"fs{sfx}")
            eng.tensor_tensor(fs[:, :, 0:16], pv[:, :, 0:16], pv[:, :, 16:32],
                              op=ALU.add)
            eng.tensor_tensor(fs[:, :, 16:24], fs[:, :, 0:8], fs[:, :, 8:16],
                              op=ALU.add)
            eng.tensor_tensor(fs[:, :, 24:28], fs[:, :, 16:20], fs[:, :, 20:24],
                              op=ALU.add)
            eng.tensor_tensor(fs[:, :, 28:30], fs[:, :, 24:26], fs[:, :, 26:28],
                              op=ALU.add)
            # final fold scatters into capG diagonal o (slots r*10+o)
            eng.tensor_tensor(
                capG[:, o:o + 10 * (n - 1) + 1:10, bt, :],
                fs[:, :, 28].rearrange("p (n k) -> p n k", k=K),
                fs[:, :, 29].rearrange("p (n k) -> p n k", k=K),
                op=ALU.add)

    # + I on the first 8 diagonal entries (slots j*10, j<8)
    nc.vector.tensor_scalar_add(
        capG[:, 0:80:10, :, :].rearrange("p j b k -> p j (b k)"),
        capG[:, 0:80:10, :, :].rearrange("p j b k -> p j (b k)"), 1.0)

    # ---------------- bordered slab LDL over [128, BK] planes
    # V lives in-place in capG slots (entry (j,i) at slot j*9+i);
    # L is compact r-major: (i,p) at rs0(p)+(i-p)
    def rs0(r):
        return r * NR - r * (r - 1) // 2

    Lbf = sb1.tile([128, 45, BK], BF16, name="Lbf")
    pivd = sb1.tile([128, NR, BK], F32, name="pivd")
    ldt = sb1.tile([128, BK], F32, name="ldt")
    nc.vector.tensor_copy(ldt[:], ldall[:].rearrange("p b k -> p (b k)"))

    def vcol(j, i0, i1):
        """V entries (j, i) for i in [i0, i1) -> [128, i1-i0, BK]"""
        return capG[:, j * 9 + i0: j * 9 + i1, :, :].rearrange(
            "p n b k -> p n (b k)")

    def vplane(j, i):
        return capG[:, j * 9 + i, :, :].rearrange("p b k -> p (b k)")

    def lcol(p, i0, i1):
        return Lbf[:, rs0(p) + (i0 - p): rs0(p) + (i1 - p), :]

    inv_cur = None
    for j in range(NR):
        nsl = NR - j
        if j > 0:
            prodscr = sbt.tile([128, j, nsl, BK], BF16, name="prodscr",
                               tag="prodscr")
            for p in range(j):
                # slab product: L(i,p) * V(p,j) for i = j..8
                nc.vector.tensor_tensor(
                    prodscr[:, p, :, :], lcol(p, j, NR),
                    _bc_mid(vplane(p, j), nsl), op=ALU.mult)
            terms = list(range(j))
            while len(terms) > 1:
                nxt = []
                for q in range(0, len(terms) - 1, 2):
                    a0, a1 = terms[q], terms[q + 1]
                    nc.gpsimd.tensor_tensor(
                        prodscr[:, a0, :, :], prodscr[:, a0, :, :],
                        prodscr[:, a1, :, :], op=ALU.add)
                    nxt.append(a0)
                if len(terms) % 2 == 1:
                    nxt.append(terms[-1])
                terms = nxt
            nc.vector.tensor_tensor(
                vcol(j, j, NR), vcol(j, j, NR), prodscr[:, terms[0], :, :],
                op=ALU.subtract)
        # pivot (f32), logdet term, inverse
        nc.vector.tensor_copy(pivd[:, j, :], vplane(j, j))
        if j < NR - 1:
            lnd = sbt.tile([128, BK], F32, name="lnd", tag="lnd")
            nc.scalar.activation(lnd[:], pivd[:, j, :], AF.Ln)
            nc.vector.tensor_tensor(ldt[:], ldt[:], lnd[:], op=ALU.add)
            inv_cur = sbt.tile([128, BK], F32, name="invj", tag="invj")
            nc.scalar.activation(inv_cur[:], lnd[:], AF.Exp, scale=-1.0)
            # L column j (rows j+1..8)
            nc.vector.tensor_tensor(
                lcol(j, j + 1, NR), vcol(j, j + 1, NR),
                _bc_mid(inv_cur[:], nsl - 1), op=ALU.mult)

    # ---------------- comp_logp, double logsumexp, local sum
    comp = sbt.tile([128, BK], F32, name="comp", tag="comp")
    nc.vector.tensor_tensor(comp[:], ldt[:], pivd[:, NR - 1, :], op=ALU.add)
    nc.vector.tensor_scalar(comp[:], comp[:], float(DIM * LOG2PI), -0.5,
                            op0=ALU.add, op1=ALU.mult)

    t_t = sbt.tile([128, NBT, K], F32, name="t_t", tag="t_t")
    nc.vector.tensor_tensor(
        t_t[:], wall[:], comp[:].rearrange("p (b k) -> p b k", k=K),
        op=ALU.add)

    def lse_k(src3d, nm):
        mx = sbt.tile([128, NBT], F32, name=f"mx{nm}", tag=f"mx{nm}")
        nc.vector.tensor_reduce(out=mx[:], in_=src3d,
                                axis=mybir.AxisListType.X, op=ALU.max)
        zs = sbt.tile([128, NBT, K], F32, name=f"zs{nm}", tag=f"zs{nm}")
        nc.vector.tensor_tensor(zs[:], src3d, _bc_inner(mx[:], K),
                                op=ALU.subtract)
        ez = sbt.tile([128, NBT, K], F32, name=f"ez{nm}", tag=f"ez{nm}")
        nc.scalar.activation(ez[:], zs[:], AF.Exp)
        sez = sbt.tile([128, NBT], F32, name=f"se{nm}", tag=f"se{nm}")
        nc.vector.tensor_reduce(out=sez[:], in_=ez[:],
                                axis=mybir.AxisListType.X, op=ALU.add)
        ls = sbt.tile([128, NBT], F32, name=f"ls{nm}", tag=f"ls{nm}")
        nc.scalar.activation(ls[:], sez[:], AF.Ln)
        out = sbt.tile([128, NBT], F32, name=f"lo{nm}", tag=f"lo{nm}")
        nc.vector.tensor_tensor(out[:], mx[:], ls[:], op=ALU.add)
        return out

    lp1 = lse_k(t_t[:], "t")
    lpw = lse_k(wall[:], "w")
    lp = sbt.tile([128, NBT], F32, name="lp", tag="lp")
    nc.vector.tensor_tensor(lp[:], lp1[:], lpw[:], op=ALU.subtract)

    lps = sbt.tile([128, 1], F32, name="lps", tag="lps")
    nc.vector.tensor_reduce(out=lps[:], in_=lp[:],
                            axis=mybir.AxisListType.X, op=ALU.add)
    ones_f = sb1.tile([128, 1], F32, name="ones_f")
    nc.vector.memset(ones_f[:], 1.0)
    pfin = ps.tile([1, 1], F32, name="pfin", tag="pbias", bufs=1)
    nc.tensor.matmul(pfin[:], lhsT=lps[:], rhs=ones_f[:], start=True, stop=True)
    yt = sbt.tile([1, 1], F32, name="yt", tag="yt")
    nc.scalar.copy(yt[:], pfin[:])
    nc.sync.dma_start(out=yout[:], in_=yt[:])

    ctx.close()


# --------------------------------------------------------------- host side

_CACHE = {}


def _perm():
    idx_w = np.arange(K)
    idx_mu = K + np.arange(K * DIM)
    base = K + K * DIM
    idx_diag = np.empty((K, DIM), np.int64)
    idx_fac = np.empty((RANK, K, DIM), np.int64)
    for k in range(K):
        blk = base + k * (DIM + DIM * RANK)
        idx_diag[k] = blk + np.arange(DIM)
        for d in range(DIM):
            for r in range(RANK):
                idx_fac[r, k, d] = blk + DIM + d * RANK + r
    return np.concatenate([idx_w, idx_mu, idx_diag.ravel(), idx_fac.ravel()])


def _prep(inputs):
    import ml_dtypes
    bf = ml_dtypes.bfloat16
    perm = _perm()
    Wp = np.asarray(inputs["Wout"], np.float32)[perm]
    bp = np.asarray(inputs["bout"], np.float32)[perm][None, :].astype(bf)
    w0t = np.ascontiguousarray(np.asarray(inputs["W0"], np.float32).T).astype(bf)
    wht = np.ascontiguousarray(
        np.transpose(np.asarray(inputs["Wh"], np.float32), (0, 2, 1))).astype(bf)
    woutt = np.ascontiguousarray(Wp.T).astype(bf)

    def v128(v):
        return np.ascontiguousarray(np.asarray(v, np.float32).reshape(4, 128).T)

    vec_list = [inputs["b0"], inputs["g0"], inputs["be0"]]
    for li in range(NL - 1):
        vec_list += [inputs["bh"][li], inputs["gh"][li], inputs["beh"][li]]
    vecs = np.stack([v128(v) for v in vec_list], axis=-1).astype(np.float32)

    data = np.asarray(inputs["data"], np.float32)
    context = np.asarray(inputs["context"], np.float32)
    in_maps = []
    for c in range(N_CORES):
        sl = slice(c * BL, (c + 1) * BL)
        in_maps.append({
            "ctxT": np.ascontiguousarray(context[sl].T).astype(bf),
            "data": np.ascontiguousarray(data[sl].reshape(NBT, 128, DIM)),
            "w0t": w0t, "wht": wht, "woutt": woutt, "boutr": bp, "vecs": vecs,
        })
    return in_maps


def kernel(**inputs):
    from concourse.bass_utils import run_bass_kernel_spmd

    if "nc" not in _CACHE:
        _CACHE["nc"] = build_program()
    nc = _CACHE["nc"]
    in_maps = _prep(inputs)
    res = run_bass_kernel_spmd(nc, in_maps, core_ids=list(range(N_CORES)))
    total = sum(float(res.results[c]["yout"][0, 0]) for c in range(N_CORES))
    return np.float32(-total / B)


# revision 14
# speedup vs baseline: 1.1185x; 1.0672x over previous
"""LowRankMixtureDensityNetwork loss on 8 Trainium2 NeuronCores.

Data-parallel over the batch (1024 rows/core), MLP weights replicated.
BatchNorm (training mode) statistics are allreduced across cores per layer.
The mixture-density tail uses a bordered 9x9 LDL factorization of
  cap~ = diag(1,..,1,0) + [A|e]^T [A|e]
whose last pivot is the Mahalanobis correction and whose first 8 log-pivots
sum to logdet(cap). Per-core partial loss sums are combined on the host.

Layout notes:
- MLP runs feature-on-partition; the output layer flips to batch-on-partition
  by using the activations as the matmul's stationary operand.
- Gram products are batched by diagonal offset o (pairs (r, r+o)) so both
  operands are dense slices of At; the d-reduction is a bf16 fold tree
  (tensor_tensor runs 2x on bf16, tensor_reduce is capped at 1x).
- cap~ is stored on a 9x9=81-slot grid: diagonal writes stride 10, column
  slabs stride 1 - all constant-stride APs.
"""
import contextlib

import numpy as np

import concourse.bass as bass
import concourse.tile as tile
from concourse import mybir
import bass_rust

F32 = mybir.dt.float32
BF16 = mybir.dt.bfloat16
AF = mybir.ActivationFunctionType
ALU = mybir.AluOpType

# problem constants
DIM, K, RANK = 32, 16, 8
CTX, H, NL, B = 128, 512, 4, 8192
OUT = K + DIM * K + (DIM + DIM * RANK) * K          # 5136
N_CORES = 8
BL = B // N_CORES                                    # 1024 rows per core
NBT = BL // 128                                      # 8 b-tiles per core
BK = NBT * K                                         # 128 (bt,k) plane width
NR = RANK + 1                                        # 9 (bordered system)
LOG2PI = float(np.log(2.0 * np.pi))

# output column regions after host-side permutation of Wout rows:
#   [w(16) | mu(k,d)(512) | diag(k,d)(512) | factor(r,k,d)(4096)]
C_W, C_MU, C_DIAG, C_FAC = 0, K, K + K * DIM, K + 2 * K * DIM

# engine-split knobs
GRAM_GPS_O = (0, 1, 2)         # Gram diagonals whose PRODUCTS run on GpSimd
ABUILD_GPS_R = (2, 3, 4, 5, 6, 7)  # A-build rows multiplied on GpSimd

# ------------------------------------------------------------- walrus quirks

_ctr = [0]


def _split_multi_waits(nc, max_waits=1):
    """walrus in this container rejects >1 sync wait per instruction; hoist
    excess waits onto same-engine NOPs placed just before the instruction."""
    n_split = 0
    for f in nc.m.functions:
        for bb in f.blocks:
            insts = bb.instructions
            out = []
            changed = False
            for inst in insts:
                si = inst.sync_info
                waits = list(si.on_wait) if si is not None else []
                if len(waits) > max_waits:
                    for w in waits[:-max_waits]:
                        _ctr[0] += 1
                        nop = mybir.InstNoOp(
                            name=f"WSPLIT-{_ctr[0]}",
                            engine=inst.engine,
                            ins=[],
                            outs=[],
                            sync_info=mybir.SyncInfo(on_wait=[w], on_update=[]),
                        )
                        out.append(nop)
                    inst.sync_info = mybir.SyncInfo(
                        on_wait=waits[-max_waits:], on_update=list(si.on_update)
                    )
                    changed = True
                    n_split += 1
                out.append(inst)
            if changed:
                bb.instructions = out
    return n_split


def _patched_drain_and_barrier(self, tick_clock, wait_clock):
    nc = self.nc
    probe = nc.sync.nop()
    wait_clock.add_sem_waits(
        probe.ins, bass_rust.ScopedClock({None: tick_clock.global_clock})
    )
    si = probe.ins.sync_info
    waits = list(si.on_wait) if si is not None else []
    if len(waits) > 1:
        probe.ins.sync_info = mybir.SyncInfo(on_wait=waits[:1], on_update=[])
        for w in waits[1:]:
            extra = nc.sync.nop()
            extra.ins.sync_info = mybir.SyncInfo(on_wait=[w], on_update=[])
    nc.sync.drain()

    nc.all_engine_barrier()
    assert self.sems is not None
    popped = nc._tile_sem_poison_stack.pop()
    assert popped is self._sem_poison
    nc.clear_and_free_semaphores(list(self.sems.allocated().values()))
    nc.all_engine_barrier()


tile.TileContext._drain_and_barrier = _patched_drain_and_barrier


def _bc_mid(ap, n):
    """[P, inner] AP -> [P, n, inner] with a stride-0 middle dim"""
    return bass.AP(tensor=ap.tensor, offset=ap.offset,
                   ap=[ap.ap[0], [0, n], ap.ap[-1]])


def _bc_inner(ap, k):
    """[P, n] AP -> [P, n, k] with a stride-0 inner dim"""
    return bass.AP(tensor=ap.tensor, offset=ap.offset,
                   ap=[ap.ap[0], ap.ap[-1], [0, k]])


# ----------------------------------------------------------------- program


def build_program(split=True):
    nc = bass.Bass("TRN2", num_devices=N_CORES)

    ctxT = nc.dram_tensor("ctxT", [CTX, BL], BF16, kind="ExternalInput")
    data = nc.dram_tensor("data", [NBT, 128, DIM], F32, kind="ExternalInput")
    w0t = nc.dram_tensor("w0t", [CTX, H], BF16, kind="ExternalInput")
    wht = nc.dram_tensor("wht", [NL - 1, H, H], BF16, kind="ExternalInput")
    woutt = nc.dram_tensor("woutt", [H, OUT], BF16, kind="ExternalInput")
    boutr = nc.dram_tensor("boutr", [1, OUT], BF16, kind="ExternalInput")
    # per-feature vectors packed [128, 4hc, 12]:
    #   0:b0 1:g0 2:be0, then per hidden l (0..2): 3+3l:bh, 4+3l:gh, 5+3l:beh
    vecs = nc.dram_tensor("vecs", [128, 4, 12], F32, kind="ExternalInput")
    yout = nc.dram_tensor("yout", [1, 1], F32, kind="ExternalOutput")

    with tile.TileContext(nc) as tc:
        _body(nc, tc, ctxT, data, w0t, wht, woutt, boutr, vecs, yout)
    if split:
        _split_multi_waits(nc)
    return nc


def _mlp(nc, tc, sb1, ps, dram, ctxT, w0t, wht, vecs):
    """feature-on-partition MLP with cross-core BN; returns u3p (bf16)."""
    ctx = contextlib.ExitStack()
    sbm = ctx.enter_context(tc.tile_pool(name="mlpwork", bufs=2))
    sbu = ctx.enter_context(tc.tile_pool(name="uacts", bufs=2))
    sbe = ctx.enter_context(tc.tile_pool(name="elu", bufs=3))

    t_ctx = sbm.tile([128, BL], BF16, name="t_ctx", tag="t_ctx", bufs=1)
    nc.sync.dma_start(out=t_ctx[:], in_=ctxT[:])
    t_w0 = sbm.tile([128, H], BF16, name="t_w0", tag="t_w0", bufs=1)
    nc.sync.dma_start(out=t_w0[:], in_=w0t[:])
    t_wh = sbm.tile([128, NL - 1, 4, H], BF16, name="t_wh", tag="t_wh", bufs=1)
    nc.sync.dma_start(out=t_wh[:], in_=wht.rearrange("l (c p) m -> p l c m", p=128))
    t_vec = sbm.tile([128, 4, 12], F32, name="t_vec", tag="t_vec", bufs=1)
    nc.sync.dma_start(out=t_vec[:], in_=vecs[:])
    eps_t = sbm.tile([128, 1], F32, name="eps_t", tag="eps_t", bufs=1)
    nc.vector.memset(eps_t[:], 1e-5)

    # collective warmup (absorb first-collective latency)
    cwu_in = dram.tile([128, 1], F32, name="cwu_in")
    cwu_out = dram.tile([128, 1], F32, name="cwu_out")
    t_junk = sbm.tile([128, 1], F32, name="t_junk", tag="t_junk", bufs=1)
    nc.vector.memset(t_junk[:], 0.0)
    nc.sync.dma_start(out=cwu_in[:], in_=t_junk[:])
    nc.gpsimd.collective_compute(
        "AllReduce", ALU.add, replica_groups=[list(range(N_CORES))],
        ins=[cwu_in[:].opt()], outs=[cwu_out[:].opt()],
    )
    t_junk2 = sbm.tile([128, 1], F32, name="t_junk2", tag="t_junk2", bufs=1)
    nc.gpsimd.dma_start(out=t_junk2[:], in_=cwu_out[:])

    u_prev = None
    u3p = None
    wfold = None
    beff = None

    for layer in range(NL):
        u_cur = sbu.tile([128, 4, BL], BF16, name=f"u{layer}", tag="u")
        nkc = 1 if layer == 0 else 4
        for hc in range(4):
            if layer == 0:
                bcol = t_vec[:, hc, 0:1]
            else:
                bcol = beff[:, hc:hc + 1]
            for bcc in range(2):
                bs = bcc * 512
                psum = ps.tile([128, 512], F32, name="zp", tag="z", bufs=3)
                for kc in range(nkc):
                    if layer == 0:
                        lhsT = t_w0[:, hc * 128:(hc + 1) * 128]
                        rhs = t_ctx[:, bs:bs + 512]
                    else:
                        lhsT = wfold[:, kc, hc * 128:(hc + 1) * 128]
                        rhs = u_prev[:, kc, bs:bs + 512]
                    nc.tensor.matmul(psum[:], lhsT=lhsT, rhs=rhs,
                                     start=(kc == 0), stop=(kc == nkc - 1))
                # ELU: u = max(z+b, min(exp(z+b)-1, 0))
                e_t = sbe.tile([128, 512], F32, name="elu_e", tag="elu_e")
                nc.scalar.activation(e_t[:], psum[:], AF.Exp, bias=bcol)
                q2 = sbe.tile([128, 512], BF16, name="elu_q", tag="elu_q")
                nc.vector.tensor_scalar(q2[:], e_t[:], -1.0, 0.0,
                                        op0=ALU.add, op1=ALU.min)
                nc.vector.scalar_tensor_tensor(
                    u_cur[:, hc, bs:bs + 512], psum[:], bcol, q2[:],
                    op0=ALU.add, op1=ALU.max)

        # ---- batch-norm stats (local) -> allreduce -> affine params
        stats = sbm.tile([128, 4, 2, 6], F32, name="bns", tag="bns")
        for hc in range(4):
            for half in range(2):
                nc.vector.bn_stats(
                    out=stats[:, hc, half, :],
                    in_=u_cur[:, hc, half * 512:(half + 1) * 512])
        mv = sbm.tile([128, 4, 2], F32, name="bnmv", tag="bnmv")
        for hc in range(4):
            nc.vector.bn_aggr(out=mv[:, hc, :], in_=stats[:, hc, :, :])
        pack = sbm.tile([128, 8], F32, name="bnp", tag="bnp")
        mm = mv[:, :, 0:1].rearrange("p h one -> p (h one)")
        vv = mv[:, :, 1:2].rearrange("p h one -> p (h one)")
        nc.vector.tensor_scalar_mul(pack[:, 0:4], mm, float(BL))
        msq = sbm.tile([128, 4], F32, name="bmsq", tag="bmsq")
        nc.vector.tensor_tensor(msq[:], mm, mm, op=ALU.mult)
        s2s = sbm.tile([128, 4], F32, name="bs2", tag="bs2")
        nc.vector.tensor_tensor(s2s[:], vv, msq[:], op=ALU.add)
        nc.vector.tensor_scalar_mul(pack[:, 4:8], s2s[:], float(BL))

        ar_in = dram.tile([128, 8], F32, name=f"arin{layer}")
        ar_out = dram.tile([128, 8], F32, name=f"arout{layer}")
        nc.sync.dma_start(out=ar_in[:], in_=pack[:])
        nc.gpsimd.collective_compute(
            "AllReduce", ALU.add, replica_groups=[list(range(N_CORES))],
            ins=[ar_in[:].opt()], outs=[ar_out[:].opt()],
        )
        red = sbm.tile([128, 8], F32, name="bnr", tag="bnr")
        nc.gpsimd.dma_start(out=red[:], in_=ar_out[:])

        iv = 0 if layer == 0 else 3 * (layer - 1) + 3
        g_col = t_vec[:, :, iv + 1]
        be_col = t_vec[:, :, iv + 2]
        m_t = sbm.tile([128, 4], F32, name="bnm", tag="bnm")
        nc.vector.tensor_scalar_mul(m_t[:], red[:, 0:4], 1.0 / B)
        msq2 = sbm.tile([128, 4], F32, name="bnm2", tag="bnm2")
        nc.vector.tensor_tensor(msq2[:], m_t[:], m_t[:], op=ALU.mult)
        var_t = sbm.tile([128, 4], F32, name="bnv", tag="bnv")
        nc.vector.scalar_tensor_tensor(
            var_t[:], red[:, 4:8], 1.0 / B, msq2[:],
            op0=ALU.mult, op1=ALU.subtract)
        # a = g * rsqrt(var+eps) = g * exp(-0.5*ln(var+eps))
        lnv = sbm.tile([128, 4], F32, name="bnl", tag="bnl")
        nc.scalar.activation(lnv[:], var_t[:], AF.Ln, bias=eps_t[:])
        rsq = sbm.tile([128, 4], F32, name="bnq", tag="bnq")
        nc.scalar.activation(rsq[:], lnv[:], AF.Exp, scale=-0.5)
        a_t = sbm.tile([128, 4], F32, name="bna", tag="bna")
        nc.vector.tensor_tensor(a_t[:], g_col, rsq[:], op=ALU.mult)
        ma = sbm.tile([128, 4], F32, name="bnma", tag="bnma")
        nc.vector.tensor_tensor(ma[:], m_t[:], a_t[:], op=ALU.mult)
        c_t = sbm.tile([128, 4], F32, name="bnc", tag="bnc")
        nc.vector.tensor_tensor(c_t[:], be_col, ma[:], op=ALU.subtract)

        if layer < NL - 1:
            # fold affine into next layer: W' = WhT * a (per contraction row)
            wfold = sbm.tile([128, 4, H], BF16, name="wf", tag="wf")
            for kc in range(4):
                nc.vector.tensor_scalar_mul(
                    wfold[:, kc, :], t_wh[:, layer, kc, :], a_t[:, kc:kc + 1])
            # bias: z_{l+1} = W'u + (Wh[layer] @ c + b_{l+1})
            c_bf = sbm.tile([128, 4], BF16, name="cbf", tag="cbf")
            nc.vector.tensor_copy(c_bf[:], c_t[:])
            beff = sbm.tile([128, 4], F32, name="beff", tag="beff")
            b_next = t_vec[:, :, 3 * layer + 3]
            for mc in range(4):
                pb = ps.tile([128, 1], F32, name="pbias", tag="pbias", bufs=1)
                for kc in range(4):
                    nc.tensor.matmul(
                        pb[:],
                        lhsT=t_wh[:, layer, kc, mc * 128:(mc + 1) * 128],
                        rhs=c_bf[:, kc:kc + 1],
                        start=(kc == 0), stop=(kc == 3))
                nc.scalar.activation(
                    beff[:, mc:mc + 1], pb[:], AF.Identity,
                    bias=b_next[:, mc:mc + 1])
            u_prev = u_cur
        else:
            # BN3 applied directly on u (Wout stays raw)
            u3p = sb1.tile([128, 4, BL], BF16, name="u3p")
            for hc in range(4):
                nc.scalar.activation(
                    u3p[:, hc, :], u_cur[:, hc, :], AF.Identity,
                    bias=c_t[:, hc:hc + 1], scale=a_t[:, hc:hc + 1])

    ctx.close()
    return u3p


def _body(nc, tc, ctxT, data, w0t, wht, woutt, boutr, vecs, yout):
    ctx = contextlib.ExitStack()
    sb1 = ctx.enter_context(tc.tile_pool(name="persist", bufs=1))
    ps = ctx.enter_context(tc.tile_pool(name="ps", bufs=1, space="PSUM"))
    dram = ctx.enter_context(tc.tile_pool(name="dram", bufs=1, space="DRAM"))

    t_wo = sb1.tile([128, 4, OUT], BF16, name="t_wo")
    nc.sync.dma_start(out=t_wo[:], in_=woutt.rearrange("(c p) m -> p c m", p=128))
    t_bout = sb1.tile([1, OUT], BF16, name="t_bout")
    nc.sync.dma_start(out=t_bout[:], in_=boutr[:])
    t_data = sb1.tile([128, NBT, DIM], F32, name="t_data")
    nc.sync.dma_start(out=t_data[:], in_=data.rearrange("b p d -> p b d"))
    ones1 = sb1.tile([1, 128], BF16, name="ones1")
    nc.vector.memset(ones1[:], 1.0)

    u3p = _mlp(nc, tc, sb1, ps, dram, ctxT, w0t, wht, vecs)

    # ---------------- output layer + mixture tail (batch-on-partition)
    sbt = ctx.enter_context(tc.tile_pool(name="tail", bufs=2))
    # cap~ on the 81-slot grid: entry (a,b), a<=b, at slot a*9+b
    capG = sb1.tile([128, 81, NBT, K], BF16, name="capG")
    ldall = sb1.tile([128, NBT, K], F32, name="ldall")
    wall = sb1.tile([128, NBT, K], F32, name="wall")

    chunks = [(C_W, K, "w"), (C_MU, K * DIM, "mu"), (C_DIAG, K * DIM, "diag")]
    chunks += [(C_FAC + r * 512, 512, f"fac{r}") for r in range(RANK)]

    FW = 16 + 8 + 4 + 2   # fold scratch cols per (pair,k): L1..L4 outputs

    for bt in range(NBT):
        bts = bt * 128
        s_f = sbt.tile([128, K * DIM], F32, name="s_f", tag="s_f")
        s_bf = sbt.tile([128, K * DIM], BF16, name="s_bf", tag="s_bf")
        diff = sbt.tile([128, K * DIM], F32, name="diff", tag="diff")
        At = sbt.tile([128, NR, 512], BF16, name="At", tag="At")

        for c0, w, kind in chunks:
            psum = ps.tile([128, w], F32, name=f"po_{kind}", tag="po", bufs=4)
            nc.tensor.matmul(psum[:], lhsT=ones1[:], rhs=t_bout[:, c0:c0 + w],
                             start=True, stop=False)
            for kc in range(4):
                nc.tensor.matmul(
                    psum[:], lhsT=u3p[:, kc, bts:bts + 128],
                    rhs=t_wo[:, kc, c0:c0 + w],
                    start=False, stop=(kc == 3))
            if kind == "w":
                nc.scalar.copy(wall[:, bt, :], psum[:])
            elif kind == "mu":
                nc.vector.tensor_tensor(
                    diff[:], _bc_mid(t_data[:, bt, :], K), psum[:],
                    op=ALU.subtract)
            elif kind == "diag":
                nc.scalar.activation(s_f[:], psum[:], AF.Exp, scale=-0.5)
                nc.vector.tensor_reduce(
                    out=ldall[:, bt, :],
                    in_=psum[:].rearrange("p (k d) -> p k d", d=DIM),
                    axis=mybir.AxisListType.X, op=ALU.add)
                nc.vector.tensor_copy(s_bf[:], s_f[:])
            else:
                r = int(kind[3:])
                # F evac on ACT (bf16), multiply by s on DVE/GPS
                fbf = sbt.tile([128, 512], BF16, name="fbf", tag="fbf", bufs=3)
                nc.scalar.copy(fbf[:], psum[:])
                eng = nc.gpsimd if r in ABUILD_GPS_R else nc.vector
                eng.tensor_tensor(At[:, r, :], fbf[:], s_bf[:], op=ALU.mult)
        nc.vector.tensor_tensor(At[:, RANK, :], diff[:], s_f[:], op=ALU.mult)

        # Gram by diagonal offset o: pairs (r, r+o), both operands dense.
        # Products of the big diagonals go to GpSimd (few large ops); all
        # folds run on DVE with contiguous outputs per level (2x mode).
        for o in range(NR):
            n = NR - o
            peng = nc.gpsimd if o in GRAM_GPS_O else nc.vector
            pscr = sbt.tile([128, n, 512], BF16, name="pscr", tag="pscr")
            peng.tensor_tensor(pscr[:], At[:, 0:n, :], At[:, o:NR, :],
                               op=ALU.mult)
            pv = pscr[:].rearrange("p n (k d) -> p (n k) d", d=DIM)
            f1 = sbt.tile([128, n * K, 16], BF16, name="f1", tag="f1")
            nc.vector.tensor_tensor(f1[:], pv[:, :, 0:16], pv[:, :, 16:32],
                                    op=ALU.add)
            f2 = sbt.tile([128, n * K, 8], BF16, name="f2", tag="f2")
            nc.vector.tensor_tensor(f2[:], f1[:, :, 0:8], f1[:, :, 8:16],
                                    op=ALU.add)
            f3 = sbt.tile([128, n * K, 4], BF16, name="f3", tag="f3")
            nc.vector.tensor_tensor(f3[:], f2[:, :, 0:4], f2[:, :, 4:8],
                                    op=ALU.add)
            f4 = sbt.tile([128, n * K, 2], BF16, name="f4", tag="f4")
            nc.vector.tensor_tensor(f4[:], f3[:, :, 0:2], f3[:, :, 2:4],
                                    op=ALU.add)
            # final fold scatters into capG diagonal o (slots r*10+o)
            nc.vector.tensor_tensor(
                capG[:, o:o + 10 * (n - 1) + 1:10, bt, :],
                f4[:, :, 0].rearrange("p (n k) -> p n k", k=K),
                f4[:, :, 1].rearrange("p (n k) -> p n k", k=K),
                op=ALU.add)

    # + I on the first 8 diagonal entries (slots j*10, j<8)
    nc.vector.tensor_scalar_add(
        capG[:, 0:80:10, :, :].rearrange("p j b k -> p j (b k)"),
        capG[:, 0:80:10, :, :].rearrange("p j b k -> p j (b k)"), 1.0)

    # ---------------- bordered slab LDL over [128, BK] planes
    # V lives in-place in capG slots (entry (j,i) at slot j*9+i);
    # L is compact r-major: (i,p) at rs0(p)+(i-p)
    def rs0(r):
        return r * NR - r * (r - 1) // 2

    Lbf = sb1.tile([128, 45, BK], BF16, name="Lbf")
    pivd = sb1.tile([128, NR, BK], F32, name="pivd")
    ldt = sb1.tile([128, BK], F32, name="ldt")
    nc.vector.tensor_copy(ldt[:], ldall[:].rearrange("p b k -> p (b k)"))

    def vcol(j, i0, i1):
        """V entries (j, i) for i in [i0, i1) -> [128, i1-i0, BK]"""
        return capG[:, j * 9 + i0: j * 9 + i1, :, :].rearrange(
            "p n b k -> p n (b k)")

    def vplane(j, i):
        return capG[:, j * 9 + i, :, :].rearrange("p b k -> p (b k)")

    def lcol(p, i0, i1):
        return Lbf[:, rs0(p) + (i0 - p): rs0(p) + (i1 - p), :]

    inv_cur = None
    for j in range(NR):
        nsl = NR - j
        if j > 0:
            prodscr = sbt.tile([128, j, nsl, BK], BF16, name="prodscr",
                               tag="prodscr")
            for p in range(j):
                # slab product: L(i,p) * V(p,j) for i = j..8
                nc.vector.tensor_tensor(
                    prodscr[:, p, :, :], lcol(p, j, NR),
                    _bc_mid(vplane(p, j), nsl), op=ALU.mult)
            terms = list(range(j))
            while len(terms) > 1:
                nxt = []
                for q in range(0, len(terms) - 1, 2):
                    a0, a1 = terms[q], terms[q + 1]
                    nc.gpsimd.tensor_tensor(
                        prodscr[:, a0, :, :], prodscr[:, a0, :, :],
                        prodscr[:, a1, :, :], op=ALU.add)
                    nxt.append(a0)
                if len(terms) % 2 == 1:
                    nxt.append(terms[-1])
                terms = nxt
            nc.vector.tensor_tensor(
                vcol(j, j, NR), vcol(j, j, NR), prodscr[:, terms[0], :, :],
                op=ALU.subtract)
        # pivot (f32), logdet term, inverse
        nc.vector.tensor_copy(pivd[:, j, :], vplane(j, j))
        if j < NR - 1:
            lnd = sbt.tile([128, BK], F32, name="lnd", tag="lnd")
            nc.scalar.activation(lnd[:], pivd[:, j, :], AF.Ln)
            nc.vector.tensor_tensor(ldt[:], ldt[:], lnd[:], op=ALU.add)
            inv_cur = sbt.tile([128, BK], F32, name="invj", tag="invj")
            nc.scalar.activation(inv_cur[:], lnd[:], AF.Exp, scale=-1.0)
            # L column j (rows j+1..8)
            nc.vector.tensor_tensor(
                lcol(j, j + 1, NR), vcol(j, j + 1, NR),
                _bc_mid(inv_cur[:], nsl - 1), op=ALU.mult)

    # ---------------- comp_logp, double logsumexp, local sum
    comp = sbt.tile([128, BK], F32, name="comp", tag="comp")
    nc.vector.tensor_tensor(comp[:], ldt[:], pivd[:, NR - 1, :], op=ALU.add)
    nc.vector.tensor_scalar(comp[:], comp[:], float(DIM * LOG2PI), -0.5,
                            op0=ALU.add, op1=ALU.mult)

    t_t = sbt.tile([128, NBT, K], F32, name="t_t", tag="t_t")
    nc.vector.tensor_tensor(
        t_t[:], wall[:], comp[:].rearrange("p (b k) -> p b k", k=K),
        op=ALU.add)

    def lse_k(src3d, nm):
        mx = sbt.tile([128, NBT], F32, name=f"mx{nm}", tag=f"mx{nm}")
        nc.vector.tensor_reduce(out=mx[:], in_=src3d,
                                axis=mybir.AxisListType.X, op=ALU.max)
        zs = sbt.tile([128, NBT, K], F32, name=f"zs{nm}", tag=f"zs{nm}")
        nc.vector.tensor_tensor(zs[:], src3d, _bc_inner(mx[:], K),
                                op=ALU.subtract)
        ez = sbt.tile([128, NBT, K], F32, name=f"ez{nm}", tag=f"ez{nm}")
        nc.scalar.activation(ez[:], zs[:], AF.Exp)
        sez = sbt.tile([128, NBT], F32, name=f"se{nm}", tag=f"se{nm}")
        nc.vector.tensor_reduce(out=sez[:], in_=ez[:],
                                axis=mybir.AxisListType.X, op=ALU.add)
        ls = sbt.tile([128, NBT], F32, name=f"ls{nm}", tag=f"ls{nm}")
        nc.scalar.activation(ls[:], sez[:], AF.Ln)
        out = sbt.tile([128, NBT], F32, name=f"lo{nm}", tag=f"lo{nm}")
        nc.vector.tensor_tensor(out[:], mx[:], ls[:], op=ALU.add)
        return out

    lp1 = lse_k(t_t[:], "t")
    lpw = lse_k(wall[:], "w")
    lp = sbt.tile([128, NBT], F32, name="lp", tag="lp")
    nc.vector.tensor_tensor(lp[:], lp1[:], lpw[:], op=ALU.subtract)

    lps = sbt.tile([128, 1], F32, name="lps", tag="lps")
    nc.vector.tensor_reduce(out=lps[:], in_=lp[:],
                            axis=mybir.AxisListType.X, op=ALU.add)
    ones_f = sb1.tile([128, 1], F32, name="ones_f")
    nc.vector.memset(ones_f[:], 1.0)
    pfin = ps.tile([1, 1], F32, name="pfin", tag="pbias", bufs=1)
    nc.tensor.matmul(pfin[:], lhsT=lps[:], rhs=ones_f[:], start=True, stop=True)
    yt = sbt.tile([1, 1], F32, name="yt", tag="yt")
    nc.scalar.copy(yt[:], pfin[:])
    nc.sync.dma_start(out=yout[:], in_=yt[:])

    ctx.close()


# --------------------------------------------------------------- host side

_CACHE = {}


def _perm():
    idx_w = np.arange(K)
    idx_mu = K + np.arange(K * DIM)
    base = K + K * DIM
    idx_diag = np.empty((K, DIM), np.int64)
    idx_fac = np.empty((RANK, K, DIM), np.int64)
    for k in range(K):
        blk = base + k * (DIM + DIM * RANK)
        idx_diag[k] = blk + np.arange(DIM)
        for d in range(DIM):
            for r in range(RANK):
                idx_fac[r, k, d] = blk + DIM + d * RANK + r
    return np.concatenate([idx_w, idx_mu, idx_diag.ravel(), idx_fac.ravel()])


def _prep(inputs):
    import ml_dtypes
    bf = ml_dtypes.bfloat16
    perm = _perm()
    Wp = np.asarray(inputs["Wout"], np.float32)[perm]
    bp = np.asarray(inputs["bout"], np.float32)[perm][None, :].astype(bf)
    w0t = np.ascontiguousarray(np.asarray(inputs["W0"], np.float32).T).astype(bf)
    wht = np.ascontiguousarray(
        np.transpose(np.asarray(inputs["Wh"], np.float32), (0, 2, 1))).astype(bf)
    woutt = np.ascontiguousarray(Wp.T).astype(bf)

    def v128(v):
        return np.ascontiguousarray(np.asarray(v, np.float32).reshape(4, 128).T)

    vec_list = [inputs["b0"], inputs["g0"], inputs["be0"]]
    for li in range(NL - 1):
        vec_list += [inputs["bh"][li], inputs["gh"][li], inputs["beh"][li]]
    vecs = np.stack([v128(v) for v in vec_list], axis=-1).astype(np.float32)

    data = np.asarray(inputs["data"], np.float32)
    context = np.asarray(inputs["context"], np.float32)
    in_maps = []
    for c in range(N_CORES):
        sl = slice(c * BL, (c + 1) * BL)
        in_maps.append({
            "ctxT": np.ascontiguousarray(context[sl].T).astype(bf),
            "data": np.ascontiguousarray(data[sl].reshape(NBT, 128, DIM)),
            "w0t": w0t, "wht": wht, "woutt": woutt, "boutr": bp, "vecs": vecs,
        })
    return in_maps


def kernel(**inputs):
    from concourse.bass_utils import run_bass_kernel_spmd

    if "nc" not in _CACHE:
        _CACHE["nc"] = build_program()
    nc = _CACHE["nc"]
    in_maps = _prep(inputs)
    res = run_bass_kernel_spmd(nc, in_maps, core_ids=list(range(N_CORES)))
    total = sum(float(res.results[c]["yout"][0, 0]) for c in range(N_CORES))
    return np.float32(-total / B)


# revision 16
# speedup vs baseline: 1.1537x; 1.0314x over previous
"""LowRankMixtureDensityNetwork loss on 8 Trainium2 NeuronCores.

Data-parallel over the batch (1024 rows/core), MLP weights replicated.
BatchNorm (training mode) statistics are allreduced across cores per layer.
The mixture-density tail uses a bordered 9x9 LDL factorization of
  cap~ = diag(1,..,1,0) + [A|e]^T [A|e]
whose last pivot is the Mahalanobis correction and whose first 8 log-pivots
sum to logdet(cap). Per-core partial loss sums are combined on the host.

Layout notes:
- MLP runs feature-on-partition; the output layer flips to batch-on-partition
  by using the activations as the matmul's stationary operand.
- Gram products are batched by diagonal offset o (pairs (r, r+o)) so both
  operands are dense slices of At; the d-reduction is a bf16 fold tree
  (tensor_tensor runs 2x on bf16, tensor_reduce is capped at 1x).
- cap~ is stored on a 9x9=81-slot grid: diagonal writes stride 10, column
  slabs stride 1 - all constant-stride APs.
"""
import contextlib

import numpy as np

import concourse.bass as bass
import concourse.tile as tile
from concourse import mybir
import bass_rust

F32 = mybir.dt.float32
BF16 = mybir.dt.bfloat16
AF = mybir.ActivationFunctionType
ALU = mybir.AluOpType

# problem constants
DIM, K, RANK = 32, 16, 8
CTX, H, NL, B = 128, 512, 4, 8192
OUT = K + DIM * K + (DIM + DIM * RANK) * K          # 5136
N_CORES = 8
BL = B // N_CORES                                    # 1024 rows per core
NBT = BL // 128                                      # 8 b-tiles per core
BK = NBT * K                                         # 128 (bt,k) plane width
NR = RANK + 1                                        # 9 (bordered system)
LOG2PI = float(np.log(2.0 * np.pi))

# output column regions after host-side permutation of Wout rows:
#   [w(16) | mu(k,d)(512) | diag(k,d)(512) | factor(r,k,d)(4096)]
C_W, C_MU, C_DIAG, C_FAC = 0, K, K + K * DIM, K + 2 * K * DIM

# engine-split knobs
GRAM_GPS_O = (0, 1, 2)         # Gram diagonals whose PRODUCTS run on GpSimd
ABUILD_GPS_R = (2, 3, 4, 5, 6, 7)  # A-build rows multiplied on GpSimd

# ------------------------------------------------------------- walrus quirks

_ctr = [0]


def _split_multi_waits(nc, max_waits=1):
    """walrus in this container rejects >1 sync wait per instruction; hoist
    excess waits onto same-engine NOPs placed just before the instruction."""
    n_split = 0
    for f in nc.m.functions:
        for bb in f.blocks:
            insts = bb.instructions
            out = []
            changed = False
            for inst in insts:
                si = inst.sync_info
                waits = list(si.on_wait) if si is not None else []
                if len(waits) > max_waits:
                    for w in waits[:-max_waits]:
                        _ctr[0] += 1
                        nop = mybir.InstNoOp(
                            name=f"WSPLIT-{_ctr[0]}",
                            engine=inst.engine,
                            ins=[],
                            outs=[],
                            sync_info=mybir.SyncInfo(on_wait=[w], on_update=[]),
                        )
                        out.append(nop)
                    inst.sync_info = mybir.SyncInfo(
                        on_wait=waits[-max_waits:], on_update=list(si.on_update)
                    )
                    changed = True
                    n_split += 1
                out.append(inst)
            if changed:
                bb.instructions = out
    return n_split


def _patched_drain_and_barrier(self, tick_clock, wait_clock):
    nc = self.nc
    probe = nc.sync.nop()
    wait_clock.add_sem_waits(
        probe.ins, bass_rust.ScopedClock({None: tick_clock.global_clock})
    )
    si = probe.ins.sync_info
    waits = list(si.on_wait) if si is not None else []
    if len(waits) > 1:
        probe.ins.sync_info = mybir.SyncInfo(on_wait=waits[:1], on_update=[])
        for w in waits[1:]:
            extra = nc.sync.nop()
            extra.ins.sync_info = mybir.SyncInfo(on_wait=[w], on_update=[])
    nc.sync.drain()

    nc.all_engine_barrier()
    assert self.sems is not None
    popped = nc._tile_sem_poison_stack.pop()
    assert popped is self._sem_poison
    nc.clear_and_free_semaphores(list(self.sems.allocated().values()))
    nc.all_engine_barrier()


tile.TileContext._drain_and_barrier = _patched_drain_and_barrier


def _bc_mid(ap, n):
    """[P, inner] AP -> [P, n, inner] with a stride-0 middle dim"""
    return bass.AP(tensor=ap.tensor, offset=ap.offset,
                   ap=[ap.ap[0], [0, n], ap.ap[-1]])


def _bc_inner(ap, k):
    """[P, n] AP -> [P, n, k] with a stride-0 inner dim"""
    return bass.AP(tensor=ap.tensor, offset=ap.offset,
                   ap=[ap.ap[0], ap.ap[-1], [0, k]])


# ----------------------------------------------------------------- program


def build_program(split=True):
    nc = bass.Bass("TRN2", num_devices=N_CORES)

    ctxT = nc.dram_tensor("ctxT", [CTX, BL], BF16, kind="ExternalInput")
    data = nc.dram_tensor("data", [NBT, 128, DIM], F32, kind="ExternalInput")
    w0t = nc.dram_tensor("w0t", [CTX, H], BF16, kind="ExternalInput")
    wht = nc.dram_tensor("wht", [NL - 1, H, H], BF16, kind="ExternalInput")
    woutt = nc.dram_tensor("woutt", [H, OUT], BF16, kind="ExternalInput")
    boutr = nc.dram_tensor("boutr", [1, OUT], BF16, kind="ExternalInput")
    # per-feature vectors packed [128, 4hc, 12]:
    #   0:b0 1:g0 2:be0, then per hidden l (0..2): 3+3l:bh, 4+3l:gh, 5+3l:beh
    vecs = nc.dram_tensor("vecs", [128, 4, 12], F32, kind="ExternalInput")
    yout = nc.dram_tensor("yout", [1, 1], F32, kind="ExternalOutput")

    with tile.TileContext(nc) as tc:
        _body(nc, tc, ctxT, data, w0t, wht, woutt, boutr, vecs, yout)
    if split:
        _split_multi_waits(nc)
    return nc


def _mlp(nc, tc, sb1, ps, dram, ctxT, w0t, wht, vecs):
    """feature-on-partition MLP with cross-core BN; returns u3p (bf16)."""
    ctx = contextlib.ExitStack()
    sbm = ctx.enter_context(tc.tile_pool(name="mlpwork", bufs=2))
    sbu = ctx.enter_context(tc.tile_pool(name="uacts", bufs=2))
    sbe = ctx.enter_context(tc.tile_pool(name="elu", bufs=3))

    t_ctx = sbm.tile([128, BL], BF16, name="t_ctx", tag="t_ctx", bufs=1)
    nc.sync.dma_start(out=t_ctx[:], in_=ctxT[:])
    t_w0 = sbm.tile([128, H], BF16, name="t_w0", tag="t_w0", bufs=1)
    nc.sync.dma_start(out=t_w0[:], in_=w0t[:])
    t_wh = sbm.tile([128, NL - 1, 4, H], BF16, name="t_wh", tag="t_wh", bufs=1)
    nc.sync.dma_start(out=t_wh[:], in_=wht.rearrange("l (c p) m -> p l c m", p=128))
    t_vec = sbm.tile([128, 4, 12], F32, name="t_vec", tag="t_vec", bufs=1)
    nc.sync.dma_start(out=t_vec[:], in_=vecs[:])
    eps_t = sbm.tile([128, 1], F32, name="eps_t", tag="eps_t", bufs=1)
    nc.vector.memset(eps_t[:], 1e-5)

    # collective warmup (absorb first-collective latency)
    cwu_in = dram.tile([128, 1], F32, name="cwu_in")
    cwu_out = dram.tile([128, 1], F32, name="cwu_out")
    t_junk = sbm.tile([128, 1], F32, name="t_junk", tag="t_junk", bufs=1)
    nc.vector.memset(t_junk[:], 0.0)
    nc.sync.dma_start(out=cwu_in[:], in_=t_junk[:])
    nc.gpsimd.collective_compute(
        "AllReduce", ALU.add, replica_groups=[list(range(N_CORES))],
        ins=[cwu_in[:].opt()], outs=[cwu_out[:].opt()],
    )
    t_junk2 = sbm.tile([128, 1], F32, name="t_junk2", tag="t_junk2", bufs=1)
    nc.gpsimd.dma_start(out=t_junk2[:], in_=cwu_out[:])

    u_prev = None
    u3p = None
    wfold = None
    beff = None

    for layer in range(NL):
        u_cur = sbu.tile([128, 4, BL], BF16, name=f"u{layer}", tag="u")
        nkc = 1 if layer == 0 else 4
        for hc in range(4):
            if layer == 0:
                bcol = t_vec[:, hc, 0:1]
            else:
                bcol = beff[:, hc:hc + 1]
            for bcc in range(2):
                bs = bcc * 512
                psum = ps.tile([128, 512], F32, name="zp", tag="z", bufs=3)
                for kc in range(nkc):
                    if layer == 0:
                        lhsT = t_w0[:, hc * 128:(hc + 1) * 128]
                        rhs = t_ctx[:, bs:bs + 512]
                    else:
                        lhsT = wfold[:, kc, hc * 128:(hc + 1) * 128]
                        rhs = u_prev[:, kc, bs:bs + 512]
                    nc.tensor.matmul(psum[:], lhsT=lhsT, rhs=rhs,
                                     start=(kc == 0), stop=(kc == nkc - 1))
                # ELU: u = max(z+b, min(exp(z+b)-1, 0))
                e_t = sbe.tile([128, 512], F32, name="elu_e", tag="elu_e")
                nc.scalar.activation(e_t[:], psum[:], AF.Exp, bias=bcol)
                q2 = sbe.tile([128, 512], BF16, name="elu_q", tag="elu_q")
                nc.vector.tensor_scalar(q2[:], e_t[:], -1.0, 0.0,
                                        op0=ALU.add, op1=ALU.min)
                nc.vector.scalar_tensor_tensor(
                    u_cur[:, hc, bs:bs + 512], psum[:], bcol, q2[:],
                    op0=ALU.add, op1=ALU.max)

        # ---- batch-norm stats (local) -> allreduce -> affine params
        stats = sbm.tile([128, 4, 2, 6], F32, name="bns", tag="bns")
        for hc in range(4):
            for half in range(2):
                nc.vector.bn_stats(
                    out=stats[:, hc, half, :],
                    in_=u_cur[:, hc, half * 512:(half + 1) * 512])
        mv = sbm.tile([128, 4, 2], F32, name="bnmv", tag="bnmv")
        for hc in range(4):
            nc.vector.bn_aggr(out=mv[:, hc, :], in_=stats[:, hc, :, :])
        pack = sbm.tile([128, 8], F32, name="bnp", tag="bnp")
        mm = mv[:, :, 0:1].rearrange("p h one -> p (h one)")
        vv = mv[:, :, 1:2].rearrange("p h one -> p (h one)")
        nc.vector.tensor_scalar_mul(pack[:, 0:4], mm, float(BL))
        msq = sbm.tile([128, 4], F32, name="bmsq", tag="bmsq")
        nc.vector.tensor_tensor(msq[:], mm, mm, op=ALU.mult)
        s2s = sbm.tile([128, 4], F32, name="bs2", tag="bs2")
        nc.vector.tensor_tensor(s2s[:], vv, msq[:], op=ALU.add)
        nc.vector.tensor_scalar_mul(pack[:, 4:8], s2s[:], float(BL))

        ar_in = dram.tile([128, 8], F32, name=f"arin{layer}")
        ar_out = dram.tile([128, 8], F32, name=f"arout{layer}")
        nc.sync.dma_start(out=ar_in[:], in_=pack[:])
        nc.gpsimd.collective_compute(
            "AllReduce", ALU.add, replica_groups=[list(range(N_CORES))],
            ins=[ar_in[:].opt()], outs=[ar_out[:].opt()],
        )
        red = sbm.tile([128, 8], F32, name="bnr", tag="bnr")
        nc.gpsimd.dma_start(out=red[:], in_=ar_out[:])

        iv = 0 if layer == 0 else 3 * (layer - 1) + 3
        g_col = t_vec[:, :, iv + 1]
        be_col = t_vec[:, :, iv + 2]
        m_t = sbm.tile([128, 4], F32, name="bnm", tag="bnm")
        nc.vector.tensor_scalar_mul(m_t[:], red[:, 0:4], 1.0 / B)
        msq2 = sbm.tile([128, 4], F32, name="bnm2", tag="bnm2")
        nc.vector.tensor_tensor(msq2[:], m_t[:], m_t[:], op=ALU.mult)
        var_t = sbm.tile([128, 4], F32, name="bnv", tag="bnv")
        nc.vector.scalar_tensor_tensor(
            var_t[:], red[:, 4:8], 1.0 / B, msq2[:],
            op0=ALU.mult, op1=ALU.subtract)
        # a = g * rsqrt(var+eps) = g * exp(-0.5*ln(var+eps))
        lnv = sbm.tile([128, 4], F32, name="bnl", tag="bnl")
        nc.scalar.activation(lnv[:], var_t[:], AF.Ln, bias=eps_t[:])
        rsq = sbm.tile([128, 4], F32, name="bnq", tag="bnq")
        nc.scalar.activation(rsq[:], lnv[:], AF.Exp, scale=-0.5)
        a_t = sbm.tile([128, 4], F32, name="bna", tag="bna")
        nc.vector.tensor_tensor(a_t[:], g_col, rsq[:], op=ALU.mult)
        ma = sbm.tile([128, 4], F32, name="bnma", tag="bnma")
        nc.vector.tensor_tensor(ma[:], m_t[:], a_t[:], op=ALU.mult)
        c_t = sbm.tile([128, 4], F32, name="bnc", tag="bnc")
        nc.vector.tensor_tensor(c_t[:], be_col, ma[:], op=ALU.subtract)

        if layer < NL - 1:
            # fold affine into next layer: W' = WhT * a (per contraction row)
            wfold = sbm.tile([128, 4, H], BF16, name="wf", tag="wf")
            for kc in range(4):
                nc.vector.tensor_scalar_mul(
                    wfold[:, kc, :], t_wh[:, layer, kc, :], a_t[:, kc:kc + 1])
            # bias: z_{l+1} = W'u + (Wh[layer] @ c + b_{l+1})
            c_bf = sbm.tile([128, 4], BF16, name="cbf", tag="cbf")
            nc.vector.tensor_copy(c_bf[:], c_t[:])
            beff = sbm.tile([128, 4], F32, name="beff", tag="beff")
            b_next = t_vec[:, :, 3 * layer + 3]
            for mc in range(4):
                pb = ps.tile([128, 1], F32, name="pbias", tag="pbias", bufs=1)
                for kc in range(4):
                    nc.tensor.matmul(
                        pb[:],
                        lhsT=t_wh[:, layer, kc, mc * 128:(mc + 1) * 128],
                        rhs=c_bf[:, kc:kc + 1],
                        start=(kc == 0), stop=(kc == 3))
                nc.scalar.activation(
                    beff[:, mc:mc + 1], pb[:], AF.Identity,
                    bias=b_next[:, mc:mc + 1])
            u_prev = u_cur
        else:
            # BN3 applied directly on u (Wout stays raw)
            u3p = sb1.tile([128, 4, BL], BF16, name="u3p")
            for hc in range(4):
                nc.scalar.activation(
                    u3p[:, hc, :], u_cur[:, hc, :], AF.Identity,
                    bias=c_t[:, hc:hc + 1], scale=a_t[:, hc:hc + 1])

    ctx.close()
    return u3p


def _body(nc, tc, ctxT, data, w0t, wht, woutt, boutr, vecs, yout):
    ctx = contextlib.ExitStack()
    sb1 = ctx.enter_context(tc.tile_pool(name="persist", bufs=1))
    ps = ctx.enter_context(tc.tile_pool(name="ps", bufs=1, space="PSUM"))
    dram = ctx.enter_context(tc.tile_pool(name="dram", bufs=1, space="DRAM"))

    t_wo = sb1.tile([128, 4, OUT], BF16, name="t_wo")
    nc.sync.dma_start(out=t_wo[:], in_=woutt.rearrange("(c p) m -> p c m", p=128))
    t_bout = sb1.tile([1, OUT], BF16, name="t_bout")
    nc.sync.dma_start(out=t_bout[:], in_=boutr[:])
    t_data = sb1.tile([128, NBT, DIM], F32, name="t_data")
    nc.sync.dma_start(out=t_data[:], in_=data.rearrange("b p d -> p b d"))
    ones1 = sb1.tile([1, 128], BF16, name="ones1")
    nc.vector.memset(ones1[:], 1.0)

    u3p = _mlp(nc, tc, sb1, ps, dram, ctxT, w0t, wht, vecs)

    # ---------------- output layer + mixture tail (batch-on-partition)
    sbt = ctx.enter_context(tc.tile_pool(name="tail", bufs=2))
    # cap~ on the 81-slot grid: entry (a,b), a<=b, at slot a*9+b
    capG = sb1.tile([128, 81, NBT, K], BF16, name="capG")
    ldall = sb1.tile([128, NBT, K], F32, name="ldall")
    wall = sb1.tile([128, NBT, K], F32, name="wall")

    chunks = [(C_W, K, "w"), (C_MU, K * DIM, "mu"), (C_DIAG, K * DIM, "diag")]
    chunks += [(C_FAC + r * 512, 512, f"fac{r}") for r in range(RANK)]

    FW = 16 + 8 + 4 + 2   # fold scratch cols per (pair,k): L1..L4 outputs

    for bt in range(NBT):
        bts = bt * 128
        s_f = sbt.tile([128, K * DIM], F32, name="s_f", tag="s_f")
        s_bf = sbt.tile([128, K * DIM], BF16, name="s_bf", tag="s_bf")
        diff = sbt.tile([128, K * DIM], F32, name="diff", tag="diff")
        At = sbt.tile([128, NR, 512], BF16, name="At", tag="At")

        for c0, w, kind in chunks:
            psum = ps.tile([128, w], F32, name=f"po_{kind}", tag="po", bufs=4)
            nc.tensor.matmul(psum[:], lhsT=ones1[:], rhs=t_bout[:, c0:c0 + w],
                             start=True, stop=False)
            for kc in range(4):
                nc.tensor.matmul(
                    psum[:], lhsT=u3p[:, kc, bts:bts + 128],
                    rhs=t_wo[:, kc, c0:c0 + w],
                    start=False, stop=(kc == 3))
            if kind == "w":
                nc.scalar.copy(wall[:, bt, :], psum[:])
            elif kind == "mu":
                nc.vector.tensor_tensor(
                    diff[:], _bc_mid(t_data[:, bt, :], K), psum[:],
                    op=ALU.subtract)
            elif kind == "diag":
                nc.scalar.activation(s_f[:], psum[:], AF.Exp, scale=-0.5)
                nc.vector.tensor_reduce(
                    out=ldall[:, bt, :],
                    in_=psum[:].rearrange("p (k d) -> p k d", d=DIM),
                    axis=mybir.AxisListType.X, op=ALU.add)
                nc.vector.tensor_copy(s_bf[:], s_f[:])
            else:
                r = int(kind[3:])
                # F evac on ACT (bf16), multiply by s on DVE/GPS
                fbf = sbt.tile([128, 512], BF16, name="fbf", tag="fbf", bufs=2)
                nc.scalar.copy(fbf[:], psum[:])
                eng = nc.gpsimd if r in ABUILD_GPS_R else nc.vector
                eng.tensor_tensor(At[:, r, :], fbf[:], s_bf[:], op=ALU.mult)
        nc.vector.tensor_tensor(At[:, RANK, :], diff[:], s_f[:], op=ALU.mult)

        # Gram by diagonal offset o: pairs (r, r+o), both operands dense.
        # Big diagonals (o in GRAM_GPS_O) multiply on GpSimd, each its own
        # buffer; the remaining diagonals multiply on DVE into one shared
        # buffer so the fold tree runs as a few large 2x ops.
        dve_os = [o for o in range(NR) if o not in GRAM_GPS_O]
        nrow_d = sum(NR - o for o in dve_os)
        pcat = sbt.tile([128, nrow_d, 512], BF16, name="pcat", tag="pcat",
                        bufs=1)
        row = 0
        groups = []
        for o in range(NR):
            n = NR - o
            if o in GRAM_GPS_O:
                pscr = sbt.tile([128, n, 512], BF16, name="pscr", tag="pscr", bufs=1)
                nc.gpsimd.tensor_tensor(
                    pscr[:], At[:, 0:n, :], At[:, o:NR, :], op=ALU.mult)
                groups.append((pscr[:], [(o, 0, n)]))
            else:
                nc.vector.tensor_tensor(
                    pcat[:, row:row + n, :], At[:, 0:n, :], At[:, o:NR, :],
                    op=ALU.mult)
                row += n
        offs = []
        row = 0
        for o in dve_os:
            offs.append((o, row, NR - o))
            row += NR - o
        groups.append((pcat[:], offs))

        for src, offlist in groups:
            m = sum(n for (_, _, n) in offlist)
            pv = src.rearrange("p n (k d) -> p (n k) d", d=DIM)
            f1 = sbt.tile([128, m * K, 16], BF16, name="f1", tag="f1", bufs=1)
            nc.vector.tensor_tensor(f1[:], pv[:, :, 0:16], pv[:, :, 16:32],
                                    op=ALU.add)
            f2 = sbt.tile([128, m * K, 8], BF16, name="f2", tag="f2", bufs=1)
            nc.vector.tensor_tensor(f2[:], f1[:, :, 0:8], f1[:, :, 8:16],
                                    op=ALU.add)
            f3 = sbt.tile([128, m * K, 4], BF16, name="f3", tag="f3", bufs=1)
            nc.vector.tensor_tensor(f3[:], f2[:, :, 0:4], f2[:, :, 4:8],
                                    op=ALU.add)
            f4 = sbt.tile([128, m * K, 2], BF16, name="f4", tag="f4", bufs=1)
            nc.vector.tensor_tensor(f4[:], f3[:, :, 0:2], f3[:, :, 2:4],
                                    op=ALU.add)
            for o, r0, n in offlist:
                nc.vector.tensor_tensor(
                    capG[:, o:o + 10 * (n - 1) + 1:10, bt, :],
                    f4[:, r0 * K:(r0 + n) * K, 0].rearrange(
                        "p (n k) -> p n k", k=K),
                    f4[:, r0 * K:(r0 + n) * K, 1].rearrange(
                        "p (n k) -> p n k", k=K),
                    op=ALU.add)

    # + I on the first 8 diagonal entries (slots j*10, j<8)
    nc.vector.tensor_scalar_add(
        capG[:, 0:80:10, :, :].rearrange("p j b k -> p j (b k)"),
        capG[:, 0:80:10, :, :].rearrange("p j b k -> p j (b k)"), 1.0)

    # ---------------- bordered slab LDL over [128, BK] planes
    # V lives in-place in capG slots (entry (j,i) at slot j*9+i);
    # L is compact r-major: (i,p) at rs0(p)+(i-p)
    def rs0(r):
        return r * NR - r * (r - 1) // 2

    Lbf = sb1.tile([128, 45, BK], BF16, name="Lbf")
    pivd = sb1.tile([128, NR, BK], F32, name="pivd")
    ldt = sb1.tile([128, BK], F32, name="ldt")
    nc.vector.tensor_copy(ldt[:], ldall[:].rearrange("p b k -> p (b k)"))

    def vcol(j, i0, i1):
        """V entries (j, i) for i in [i0, i1) -> [128, i1-i0, BK]"""
        return capG[:, j * 9 + i0: j * 9 + i1, :, :].rearrange(
            "p n b k -> p n (b k)")

    def vplane(j, i):
        return capG[:, j * 9 + i, :, :].rearrange("p b k -> p (b k)")

    def lcol(p, i0, i1):
        return Lbf[:, rs0(p) + (i0 - p): rs0(p) + (i1 - p), :]

    inv_cur = None
    for j in range(NR):
        nsl = NR - j
        if j > 0:
            prodscr = sbt.tile([128, j, nsl, BK], BF16, name="prodscr",
                               tag="prodscr")
            for p in range(j):
                # slab product: L(i,p) * V(p,j) for i = j..8
                nc.vector.tensor_tensor(
                    prodscr[:, p, :, :], lcol(p, j, NR),
                    _bc_mid(vplane(p, j), nsl), op=ALU.mult)
            terms = list(range(j))
            while len(terms) > 1:
                nxt = []
                for q in range(0, len(terms) - 1, 2):
                    a0, a1 = terms[q], terms[q + 1]
                    nc.gpsimd.tensor_tensor(
                        prodscr[:, a0, :, :], prodscr[:, a0, :, :],
                        prodscr[:, a1, :, :], op=ALU.add)
                    nxt.append(a0)
                if len(terms) % 2 == 1:
                    nxt.append(terms[-1])
                terms = nxt
            nc.vector.tensor_tensor(
                vcol(j, j, NR), vcol(j, j, NR), prodscr[:, terms[0], :, :],
                op=ALU.subtract)
        # pivot (f32), logdet term, inverse
        nc.vector.tensor_copy(pivd[:, j, :], vplane(j, j))
        if j < NR - 1:
            lnd = sbt.tile([128, BK], F32, name="lnd", tag="lnd")
            nc.scalar.activation(lnd[:], pivd[:, j, :], AF.Ln)
            nc.vector.tensor_tensor(ldt[:], ldt[:], lnd[:], op=ALU.add)
            inv_cur = sbt.tile([128, BK], F32, name="invj", tag="invj")
            nc.scalar.activation(inv_cur[:], lnd[:], AF.Exp, scale=-1.0)
            # L column j (rows j+1..8)
            nc.vector.tensor_tensor(
                lcol(j, j + 1, NR), vcol(j, j + 1, NR),
                _bc_mid(inv_cur[:], nsl - 1), op=ALU.mult)

    # ---------------- comp_logp, double logsumexp, local sum
    comp = sbt.tile([128, BK], F32, name="comp", tag="comp")
    nc.vector.tensor_tensor(comp[:], ldt[:], pivd[:, NR - 1, :], op=ALU.add)
    nc.vector.tensor_scalar(comp[:], comp[:], float(DIM * LOG2PI), -0.5,
                            op0=ALU.add, op1=ALU.mult)

    t_t = sbt.tile([128, NBT, K], F32, name="t_t", tag="t_t")
    nc.vector.tensor_tensor(
        t_t[:], wall[:], comp[:].rearrange("p (b k) -> p b k", k=K),
        op=ALU.add)

    def lse_k(src3d, nm):
        mx = sbt.tile([128, NBT], F32, name=f"mx{nm}", tag=f"mx{nm}")
        nc.vector.tensor_reduce(out=mx[:], in_=src3d,
                                axis=mybir.AxisListType.X, op=ALU.max)
        zs = sbt.tile([128, NBT, K], F32, name=f"zs{nm}", tag=f"zs{nm}")
        nc.vector.tensor_tensor(zs[:], src3d, _bc_inner(mx[:], K),
                                op=ALU.subtract)
        ez = sbt.tile([128, NBT, K], F32, name=f"ez{nm}", tag=f"ez{nm}")
        nc.scalar.activation(ez[:], zs[:], AF.Exp)
        sez = sbt.tile([128, NBT], F32, name=f"se{nm}", tag=f"se{nm}")
        nc.vector.tensor_reduce(out=sez[:], in_=ez[:],
                                axis=mybir.AxisListType.X, op=ALU.add)
        ls = sbt.tile([128, NBT], F32, name=f"ls{nm}", tag=f"ls{nm}")
        nc.scalar.activation(ls[:], sez[:], AF.Ln)
        out = sbt.tile([128, NBT], F32, name=f"lo{nm}", tag=f"lo{nm}")
        nc.vector.tensor_tensor(out[:], mx[:], ls[:], op=ALU.add)
        return out

    lp1 = lse_k(t_t[:], "t")
    lpw = lse_k(wall[:], "w")
    lp = sbt.tile([128, NBT], F32, name="lp", tag="lp")
    nc.vector.tensor_tensor(lp[:], lp1[:], lpw[:], op=ALU.subtract)

    lps = sbt.tile([128, 1], F32, name="lps", tag="lps")
    nc.vector.tensor_reduce(out=lps[:], in_=lp[:],
                            axis=mybir.AxisListType.X, op=ALU.add)
    ones_f = sb1.tile([128, 1], F32, name="ones_f")
    nc.vector.memset(ones_f[:], 1.0)
    pfin = ps.tile([1, 1], F32, name="pfin", tag="pbias", bufs=1)
    nc.tensor.matmul(pfin[:], lhsT=lps[:], rhs=ones_f[:], start=True, stop=True)
    yt = sbt.tile([1, 1], F32, name="yt", tag="yt")
    nc.scalar.copy(yt[:], pfin[:])
    nc.sync.dma_start(out=yout[:], in_=yt[:])

    ctx.close()


# --------------------------------------------------------------- host side

_CACHE = {}


def _perm():
    idx_w = np.arange(K)
    idx_mu = K + np.arange(K * DIM)
    base = K + K * DIM
    idx_diag = np.empty((K, DIM), np.int64)
    idx_fac = np.empty((RANK, K, DIM), np.int64)
    for k in range(K):
        blk = base + k * (DIM + DIM * RANK)
        idx_diag[k] = blk + np.arange(DIM)
        for d in range(DIM):
            for r in range(RANK):
                idx_fac[r, k, d] = blk + DIM + d * RANK + r
    return np.concatenate([idx_w, idx_mu, idx_diag.ravel(), idx_fac.ravel()])


def _prep(inputs):
    import ml_dtypes
    bf = ml_dtypes.bfloat16
    perm = _perm()
    Wp = np.asarray(inputs["Wout"], np.float32)[perm]
    bp = np.asarray(inputs["bout"], np.float32)[perm][None, :].astype(bf)
    w0t = np.ascontiguousarray(np.asarray(inputs["W0"], np.float32).T).astype(bf)
    wht = np.ascontiguousarray(
        np.transpose(np.asarray(inputs["Wh"], np.float32), (0, 2, 1))).astype(bf)
    woutt = np.ascontiguousarray(Wp.T).astype(bf)

    def v128(v):
        return np.ascontiguousarray(np.asarray(v, np.float32).reshape(4, 128).T)

    vec_list = [inputs["b0"], inputs["g0"], inputs["be0"]]
    for li in range(NL - 1):
        vec_list += [inputs["bh"][li], inputs["gh"][li], inputs["beh"][li]]
    vecs = np.stack([v128(v) for v in vec_list], axis=-1).astype(np.float32)

    data = np.asarray(inputs["data"], np.float32)
    context = np.asarray(inputs["context"], np.float32)
    in_maps = []
    for c in range(N_CORES):
        sl = slice(c * BL, (c + 1) * BL)
        in_maps.append({
            "ctxT": np.ascontiguousarray(context[sl].T).astype(bf),
            "data": np.ascontiguousarray(data[sl].reshape(NBT, 128, DIM)),
            "w0t": w0t, "wht": wht, "woutt": woutt, "boutr": bp, "vecs": vecs,
        })
    return in_maps


def kernel(**inputs):
    from concourse.bass_utils import run_bass_kernel_spmd

    if "nc" not in _CACHE:
        _CACHE["nc"] = build_program()
    nc = _CACHE["nc"]
    in_maps = _prep(inputs)
    res = run_bass_kernel_spmd(nc, in_maps, core_ids=list(range(N_CORES)))
    total = sum(float(res.results[c]["yout"][0, 0]) for c in range(N_CORES))
    return np.float32(-total / B)
